# revision 51
# baseline (speedup 1.0000x reference)
"""Trainium2 Bass kernel for nn_DUP_block_90391881712206.

Math per (k,b) via Woodbury (no 16x16 inversions):
    GF = conj(Gh)^T @ Frf            [Ng,Nrf]
    T' = Lam @ GF
    S  = I + u * GF^H @ T'  (u = 1/beta)
    Y  = inv(S)  (4x4 complex, 2x2 block Schur, f32)
    V  = u * T' @ Y
    C  = T'^H @ V ; D = u * Y @ C
    E  = T' @ D ; W = Lam @ V ; mgf = W - E
    eg_k = Gh @ mgf
eg = -mean_k(eg_k) * P_mask, then Riemannian step + unit-disk clamp.

Implementation: data-parallel over B across 8 cores (256 b/core, 2 blocks
of 128 partitions). Batched small complex matmuls on DVE in fp16: one
broadcast-AP multiply builds the product tensor [outc,d1,d2,c,j] (packed
last dim -> 2x DVE mode), then log2 halving-fold adds reduce segments
(packed -> 2x). Host pre-lays inputs (conj baked into G's imag for GF).
"""

import numpy as np
import sys
from contextlib import ExitStack

sys.path.insert(0, "/opt/trn_rl_repo")

import concourse.bass as bass
import concourse.bacc as bacc_mod
import concourse.tile as tile
from concourse import mybir

Nk, B, Nt, Nrf, Ng = 16, 2048, 32, 4, 16
NCORES = 8
BL = B // NCORES
P = 128
H = BL // P
KC = 2                     # k per chunk
NCH = Nk // KC
BN_EPS = 1e-3
ALPHA = 0.1

F16 = mybir.dt.float16
F32 = mybir.dt.float32
AX = mybir.AxisListType.X
ADD = mybir.AluOpType.add
SUB = mybir.AluOpType.subtract
MULT = mybir.AluOpType.mult
AF = mybir.ActivationFunctionType

# per-k widths (fp16 elems)
WG1 = Ng * 2 * Nt          # 1024  g1 [g,c,n]
WG2 = Nt * 2 * Ng          # 1024  g2 [n,c,g]
WLAM = Ng * 2 * Ng         # 512   lam [g,c,g']


def av(t, off, dims):
    """AP view of tile t at free-offset `off` with free dims [[stride,n],..]."""
    return bass.AP(tensor=t.tensor, offset=t.offset + off,
                   ap=[list(t.ap[0])] + [list(d) for d in dims])


def build_nc(debug=False):
    nc = bacc_mod.Bacc()
    gpe = nc.dram_tensor("gpe", [H, P, 64 * Nk * Ng], F16, kind="ExternalInput")
    fpe = nc.dram_tensor("fpe", [H, P, 64 * 16], F16, kind="ExternalInput")
    stg = nc.dram_tensor("stg", [H, 16, P, 256], F16, kind="Internal")
    g3 = nc.dram_tensor("g3", [H, P, 32 * Nk * Nt], F16, kind="ExternalInput")
    mstg = nc.dram_tensor("mstg", [H, Nk, P, 256], F16, kind="Internal")
    estg = nc.dram_tensor("estg", [H, P, 1024], F32, kind="Internal")
    lam = nc.dram_tensor("lam", [Nk, BL, WLAM], F16, kind="ExternalInput")
    fb4 = nc.dram_tensor("fb4", [BL, 2 * Nrf * 2 * Nt], F16, kind="ExternalInput")
    f32t = nc.dram_tensor("f32t", [BL, 2 * Nt * Nrf], F32, kind="ExternalInput")
    u32 = nc.dram_tensor("u32", [BL, Nk], F32, kind="ExternalInput")
    aux = nc.dram_tensor("aux", [5 * 2 * Nt + 1 + Nt * Nrf], F32, kind="ExternalInput")
    out = nc.dram_tensor("out", [BL, Nt * Nrf * 2], F32, kind="ExternalOutput")
    dbg = None
    if debug:
        dbg = {nm: nc.dram_tensor("dbg_" + nm, [BL, w], dt, kind="ExternalOutput")
               for nm, w, dt in (("a3s", Nk * 128, F16), ("a3ve", Nk * 128, F16),
                                 ("S32", Nk * 32, F32), ("Y32", Nk * 32, F32),
                                 ("vw", Nk * 256, F16), ("keg", Nk * 256, F16),
                                 ("egT", 256, F32))}

    with ExitStack() as ctx:
        tc = ctx.enter_context(tile.TileContext(nc))
        kern(ctx, tc, gpe, fpe, stg, g3, mstg, estg, lam, fb4, f32t, u32, aux,
             out, dbg)
    if not nc.is_finalized():
        nc.finalize()
    return nc


def kern(ctx, tc, gpe, fpe, stg, g3, mstg, estg, lam, fb4, f32t, u32, aux,
         out, dbg=None):
    nc = tc.nc
    v = nc.vector

    singles = ctx.enter_context(tc.tile_pool(name="singles", bufs=1))
    loads = ctx.enter_context(tc.tile_pool(name="loads", bufs=3))
    blk = ctx.enter_context(tc.tile_pool(name="blk", bufs=1))
    scr = ctx.enter_context(tc.tile_pool(name="scr", bufs=1))
    psum = ctx.enter_context(tc.tile_pool(name="psum", bufs=4, space="PSUM"))

    # ---- broadcast-load aux params ----
    aux_t = singles.tile([P, 449], F32, name="aux_bc")
    aux_ap = aux[:]
    src = bass.AP(tensor=aux_ap.tensor, offset=aux_ap.offset,
                  ap=[[0, P]] + list(aux_ap.ap))
    nc.gpsimd.dma_start(out=aux_t, in_=src)
    gam_t, bb_t = aux_t[:, 0:64], aux_t[:, 64:128]
    bm_t, bv_t = aux_t[:, 128:192], aux_t[:, 192:256]
    dw_t, db_t = aux_t[:, 256:320], aux_t[:, 320:321]
    pm_t = aux_t[:, 321:449]

    zero1 = singles.tile([P, 1], F32, name="zero1")
    neg1 = singles.tile([P, 1], F32, name="neg1")
    v.memset(zero1, 0.0)
    v.memset(neg1, -1.0)
    eye16 = singles.tile([P, 16], F32, name="eye16")
    v.memset(eye16, 0.0)
    v.memset(av(eye16, 0, [[5, 4]]), 1.0)

    # bn scale/shift
    bnsc = singles.tile([P, 64], F32, name="bnsc")
    bnsh = singles.tile([P, 64], F32, name="bnsh")
    v.tensor_scalar_add(bnsc, bv_t, BN_EPS)
    v.reciprocal(bnsc, bnsc)
    nc.scalar.activation(bnsc, bnsc, AF.Sqrt, bias=zero1)
    v.tensor_mul(bnsc, bnsc, gam_t)
    v.tensor_mul(bnsh, bm_t, bnsc)
    v.tensor_sub(bnsh, bb_t, bnsh)

    # ---- scratch tiles ----
    # product tensors + fold ping-pong (sized for biggest cmat: 4w=8192/k)
    Pt = scr.tile([P, KC * 4096], F16, name="Pt")
    Pf1 = scr.tile([P, KC * 2048], F16, name="Pf1")
    Pf2 = scr.tile([P, KC * 1024], F16, name="Pf2")
    sc0 = scr.tile([P, KC * 256], F32, name="sc0")   # f32 scratch (D etc)
    sc1 = scr.tile([P, 256], F32, name="sc1")
    sc2 = scr.tile([P, 256], F32, name="sc2")

    def fold_reduce(src_t, src_off, nseg, L, eng1=None):
        """Sum contiguous segments of length L (pow2) via halving adds.
        Returns (tile, offset) of compact [nseg] result. eng1: engine for
        the first (widest) fold level (e.g. nc.gpsimd to offload)."""
        cur_t, cur_off, cl = src_t, src_off, L
        ping = [Pf1, Pf2]
        pi = 0
        first = True
        while cl > 1:
            half = cl // 2
            dst = ping[pi]
            pi ^= 1
            eng = eng1 if (first and eng1 is not None) else v
            eng.tensor_add(av(dst, 0, [[half, nseg], [1, half]]),
                           av(cur_t, cur_off, [[cl, nseg], [1, half]]),
                           av(cur_t, cur_off + half, [[cl, nseg], [1, half]]))
            cur_t, cur_off, cl = dst, 0, half
            first = False
        return cur_t, cur_off

    def cmat(a_t, a_off, a_d1s, a_cs, b4_t, b4_off, b4_ocs, b4_d2s, b4_cs,
             d1, d2, j, p_off=0):
        """Product P[outc,d1,d2,c,j] = A[d1,c,j] * B4[outc,d2,c,j], then
        fold-reduce (c,j) -> K [outc,d1,d2] compact fp16. One k at a time
        for the mul (4-dim APs); returns nothing (call fold separately)."""
        L = 2 * j
        w2 = d1 * d2 * L
        a_v = av(a_t, a_off, [[a_d1s, d1], [0, d2], [1, L]])
        for oc in range(2):
            ov = av(Pt, p_off + oc * w2, [[d2 * L, d1], [L, d2], [1, L]])
            b_v = av(b4_t, b4_off + oc * b4_ocs, [[0, d1], [b4_d2s, d2], [1, L]])
            v.tensor_mul(ov, a_v, b_v)

    # ============ per-block main ============
    out_v = out
    for h in range(H):
        hs = h * P

        fb4_t = blk.tile([P, 512], F16, name="fb4_t")
        f32_t = blk.tile([P, 256], F32, name="f32_t")
        u_t = blk.tile([P, Nk], F32, name="u_t")
        nc.sync.dma_start(out=fb4_t, in_=fb4[hs:hs + P])
        nc.sync.dma_start(out=f32_t, in_=f32t[hs:hs + P])
        nc.sync.dma_start(out=u_t, in_=u32[hs:hs + P])
        fre = f32_t[:, 0:128]
        fim = f32_t[:, 128:256]

        lam_t = blk.tile([P, Nk * WLAM], F16, name="lam_t")
        nc.sync.dma_start(out=lam_t.rearrange("p (k w) -> p k w", k=Nk),
                          in_=lam[:, hs:hs + P].rearrange("k b w -> b k w"))

        # all-k mid tensors (fp16 per-k layouts)
        a3ve = blk.tile([P, Nk * 128], F16, name="a3ve")  # T' [g,c,r]
        a3c = blk.tile([P, Nk * 128], F16, name="a3c")    # conj(T') [r,c,g']
        b4s = blk.tile([P, Nk * 256], F16, name="b4s")    # B4(T') [oc,r2,c,g']
        a3s = blk.tile([P, Nk * 128], F16, name="a3s")    # conj(GF) [r,c,g']
        b4t = blk.tile([P, Nk * 256], F16, name="b4t")    # B4(GF) [oc,r,c,g']
        S32 = blk.tile([P, Nk * 32], F32, name="S32")     # [k,c,i,j]
        Y32 = blk.tile([P, Nk * 32], F32, name="Y32")
        Yu32 = blk.tile([P, Nk * 32], F32, name="Yu32")
        b4y = blk.tile([P, Nk * 64], F16, name="b4y")     # B4(Yu) [oc,r2,c,r]
        b4cw = blk.tile([P, KC * 256], F16, name="b4cw")  # B4(V) [oc,r2,c,g']
        C32 = blk.tile([P, KC * 32], F32, name="C32")
        D32 = blk.tile([P, KC * 32], F32, name="D32")
        b4e = blk.tile([P, KC * 64], F16, name="b4e")     # B4(D) [oc,r2,c,r]
        ke = blk.tile([P, KC * 128], F16, name="ke")      # saved K of E
        egT = blk.tile([P, 256], F32, name="egT")
        mgf_t = blk.tile([P, Nk * 256], F16, name="mgf_t")

        # ---------- PE: GF for all k ----------
        # stationary fpe [128=(s,cc,n), 16=(s,r,oc)] per pair (block-diag Frf);
        # moving gmov [128, 256=(k,g)] per pair; out psum [16, 256] per pair.
        gmov = blk.tile([P, 64 * Nk * Ng], F16, name="gmov")
        fpe_t = blk.tile([P, 64 * 16], F16, name="fpe_t")
        nc.sync.dma_start(out=gmov, in_=gpe[h])
        nc.sync.dma_start(out=fpe_t, in_=fpe[h])
        for grp in range(16):
            pt = psum.tile([P, 256], F32, name="pgf")
            for qq in range(4):
                pair = grp * 4 + qq
                nc.tensor.matmul(out=pt[32 * qq:32 * qq + 16, :],
                                 lhsT=fpe_t[:, pair * 16:(pair + 1) * 16],
                                 rhs=av(gmov, pair * 256, [[1, 256]]),
                                 start=True, stop=True,
                                 tile_position=(0, 32 * qq))
            sgf = loads.tile([P, 256], F16, name="sgf")
            nc.scalar.activation(sgf, pt, AF.Copy)
            nc.sync.dma_start(out=stg[h, grp], in_=sgf)
        # stg row (per grp): 32*qq + 8s + 2r + oc ; col = k*16+g.
        # gather to a3s [p=sample, (r,c,k,g')]: 1 DMA per s
        st0 = stg[h, 0]
        pstride = Nk * 128
        for s in (0, 1):
            for r in range(4):
                for c in range(2):
                    nc.sync.dma_start(
                        out=bass.AP(tensor=a3s.tensor,
                                    offset=a3s.offset + s * pstride + r * 32 + c * 16,
                                    ap=[[2 * pstride, 64], [128, 16], [1, 16]]),
                        in_=bass.AP(tensor=st0.tensor,
                                    offset=st0.offset + s * 2048 + r * 512 + c * 256,
                                    ap=[[8192, 64], [16, 16], [1, 16]]))

        # b4t = B4(GF) from a3s(=conj GF): (0,0)=re=a3s0,(0,1)=-im=a3s1,
        # (1,0)=im=-a3s1,(1,1)=re=a3s0   [k-batched, all k]
        v.tensor_copy(av(b4t, 0, [[256, Nk], [32, 4], [1, 16]]),
                      av(a3s, 0, [[128, Nk], [32, 4], [1, 16]]))
        v.tensor_copy(av(b4t, 16, [[256, Nk], [32, 4], [1, 16]]),
                      av(a3s, 16, [[128, Nk], [32, 4], [1, 16]]))
        v.tensor_scalar_mul(av(b4t, 128, [[256, Nk], [32, 4], [1, 16]]),
                            av(a3s, 16, [[128, Nk], [32, 4], [1, 16]]), -1.0)
        v.tensor_copy(av(b4t, 144, [[256, Nk], [32, 4], [1, 16]]),
                      av(a3s, 0, [[128, Nk], [32, 4], [1, 16]]))

        # ---------- pass 1: T', S per k-chunk ----------
        for ch in range(NCH):
            k0 = ch * KC

            # T' = Lam @ GF: A=lam [g,c,g'], B4=b4t; d1=g,d2=r,j=g'
            for kk in range(KC):
                cmat(lam_t, (k0 + kk) * WLAM, 32, 16, b4t, (k0 + kk) * 256, 128, 32, 16,
                     Ng, Nrf, Ng, p_off=kk * 4096)
            kt, ko = fold_reduce(Pt, 0, KC * 2 * Ng * Nrf, 32)
            # K [kk, outc, g, r] strides 128,64,4,1
            for kk in range(KC):
                kb = ko + kk * 128
                kg = k0 + kk
                # a3ve = T' [g,c,r]: direct
                v.tensor_copy(av(a3ve, kg * 128, [[8, 16], [1, 4]]),
                              av(kt, kb, [[4, 16], [1, 4]]))
                v.tensor_copy(av(a3ve, kg * 128 + 4, [[8, 16], [1, 4]]),
                              av(kt, kb + 64, [[4, 16], [1, 4]]))
                # a3c = conj(T') [r,c,g']
                v.tensor_copy(av(a3c, kg * 128, [[32, 4], [1, 16]]),
                              av(kt, kb, [[1, 4], [4, 16]]))
                v.tensor_scalar_mul(av(a3c, kg * 128 + 16, [[32, 4], [1, 16]]),
                                    av(kt, kb + 64, [[1, 4], [4, 16]]), -1.0)
                # b4s = B4(T') [oc,r2,c,g']
                v.tensor_copy(av(b4s, kg * 256, [[32, 4], [1, 16]]),
                              av(kt, kb, [[1, 4], [4, 16]]))
                v.tensor_scalar_mul(av(b4s, kg * 256 + 16, [[32, 4], [1, 16]]),
                                    av(kt, kb + 64, [[1, 4], [4, 16]]), -1.0)
                v.tensor_copy(av(b4s, kg * 256 + 128, [[32, 4], [1, 16]]),
                              av(kt, kb + 64, [[1, 4], [4, 16]]))
                v.tensor_copy(av(b4s, kg * 256 + 144, [[32, 4], [1, 16]]),
                              av(kt, kb, [[1, 4], [4, 16]]))

            # S' = GF^H T': A=a3s [r1,c,g'], B4=b4s; d1=r1,d2=r2,j=g'
            for kk in range(KC):
                cmat(a3s, (k0 + kk) * 128, 32, 16, b4s, (k0 + kk) * 256, 128, 32, 16,
                     Nrf, Nrf, Ng, p_off=kk * 1024)
            kt, ko = fold_reduce(Pt, 0, KC * 2 * Nrf * Nrf, 32)
            # K [kk, outc, r1, r2] strides 32,16,4,1 -> S32 [k,c,i,j]
            v.tensor_copy(av(S32, k0 * 32, [[32, KC], [16, 2], [1, 16]]),
                          av(kt, ko, [[32, KC], [16, 2], [1, 16]]))
            # S = I + u*S'
            v.tensor_mul(av(S32, k0 * 32, [[32, KC], [1, 32]]),
                         av(S32, k0 * 32, [[32, KC], [1, 32]]),
                         av(u_t, k0, [[1, KC], [0, 32]]))
            v.tensor_add(av(S32, k0 * 32, [[32, KC], [1, 16]]),
                         av(S32, k0 * 32, [[32, KC], [1, 16]]),
                         av(eye16, 0, [[0, KC], [1, 16]]))

        if dbg is not None:
            nc.sync.dma_start(out=dbg["a3s"][hs:hs + P], in_=a3s)
            nc.sync.dma_start(out=dbg["a3ve"][hs:hs + P], in_=a3ve)
            nc.sync.dma_start(out=dbg["S32"][hs:hs + P], in_=S32)

        # ---------- pass 2: batched 4x4 inversion (2x2 Schur), f32 ----------
        inv4_batched(v, scr, S32, Y32)
        # Yu = u*Y
        v.tensor_mul(av(Yu32, 0, [[32, Nk], [1, 32]]),
                     av(Y32, 0, [[32, Nk], [1, 32]]),
                     av(u_t, 0, [[1, Nk], [0, 32]]))
        # b4y = B4(Yu) [oc,r2,c,r], B[r2,r]=Yu[r,r2] (transposed)
        for (qoff, soff, sgn) in ((0, 0, 1.0), (4, 16, -1.0),
                                  (32, 16, 1.0), (36, 0, 1.0)):
            if sgn > 0:
                v.tensor_copy(av(b4y, qoff, [[64, Nk], [8, 4], [1, 4]]),
                              av(Yu32, soff, [[32, Nk], [1, 4], [4, 4]]))
            else:
                v.tensor_scalar_mul(av(b4y, qoff, [[64, Nk], [8, 4], [1, 4]]),
                                    av(Yu32, soff, [[32, Nk], [1, 4], [4, 4]]), -1.0)

        if dbg is not None:
            nc.sync.dma_start(out=dbg["Y32"][hs:hs + P], in_=Y32)

        # ---------- pass 3: V, C, D, E, W, mgf, eg per k-chunk ----------
        for ch in range(NCH):
            k0 = ch * KC

            # V = T' @ Yu: A=a3ve [g,c,r], B4=b4y; d1=g,d2=r2,j=r
            for kk in range(KC):
                cmat(a3ve, (k0 + kk) * 128, 8, 4, b4y, (k0 + kk) * 64, 32, 8, 4,
                     Ng, Nrf, Nrf, p_off=kk * 1024)
            kt, ko = fold_reduce(Pt, 0, KC * 2 * Ng * Nrf, 8)
            # K [kk,outc,g,r2] strides 128,64,4,1 -> b4cw [oc,r2,c,g'](V)
            for kk in range(KC):
                kb = ko + kk * 128
                v.tensor_copy(av(b4cw, kk * 256, [[32, 4], [1, 16]]),
                              av(kt, kb, [[1, 4], [4, 16]]))
                v.tensor_scalar_mul(av(b4cw, kk * 256 + 16, [[32, 4], [1, 16]]),
                                    av(kt, kb + 64, [[1, 4], [4, 16]]), -1.0)
                v.tensor_copy(av(b4cw, kk * 256 + 128, [[32, 4], [1, 16]]),
                              av(kt, kb + 64, [[1, 4], [4, 16]]))
                v.tensor_copy(av(b4cw, kk * 256 + 144, [[32, 4], [1, 16]]),
                              av(kt, kb, [[1, 4], [4, 16]]))

            if dbg is not None:
                nc.sync.dma_start(out=dbg["vw"][hs:hs + P, k0 * 256:(k0 + KC) * 256],
                                  in_=b4cw)

            # C = T'^H @ V: A=a3c [r1,c,g'], B4=b4cw; d1=r1,d2=r2,j=g'
            for kk in range(KC):
                cmat(a3c, (k0 + kk) * 128, 32, 16, b4cw, kk * 256, 128, 32, 16,
                     Nrf, Nrf, Ng, p_off=kk * 1024)
            kt, ko = fold_reduce(Pt, 0, KC * 2 * Nrf * Nrf, 32)
            v.tensor_copy(av(C32, 0, [[32, KC], [16, 2], [1, 16]]),
                          av(kt, ko, [[32, KC], [16, 2], [1, 16]]))

            # D = u * Y @ C (f32). product dims per k: [r1, r2, m]
            for comp in (0, 1):
                for kk in range(KC):
                    yv = lambda c: av(Y32, (k0 + kk) * 32 + c * 16,
                                      [[4, 4], [0, 4], [1, 4]])
                    cv = lambda c: av(C32, kk * 32 + c * 16,
                                      [[0, 4], [1, 4], [4, 4]])
                    t0 = av(sc0, kk * 64, [[16, 4], [4, 4], [1, 4]])
                    t1 = av(sc0, KC * 64 + kk * 64, [[16, 4], [4, 4], [1, 4]])
                    v.tensor_mul(t0, yv(0), cv(comp))
                    v.tensor_mul(t1, yv(1), cv(1 - comp))
                t0f = av(sc0, 0, [[1, KC * 64]])
                t1f = av(sc0, KC * 64, [[1, KC * 64]])
                if comp == 0:
                    v.tensor_sub(t0f, t0f, t1f)
                else:
                    v.tensor_add(t0f, t0f, t1f)
                v.tensor_reduce(av(D32, comp * 16, [[32, KC], [1, 16], [0, 1]]),
                                av(sc0, 0, [[64, KC], [4, 16], [1, 4]]),
                                axis=AX, op=ADD)
            v.tensor_mul(av(D32, 0, [[32, KC], [1, 32]]),
                         av(D32, 0, [[32, KC], [1, 32]]),
                         av(u_t, k0, [[1, KC], [0, 32]]))
            # b4e = B4(D) [oc,r2,c,r], B[r2,r]=D[r,r2]
            for (qoff, soff, sgn) in ((0, 0, 1.0), (4, 16, -1.0),
                                      (32, 16, 1.0), (36, 0, 1.0)):
                if sgn > 0:
                    v.tensor_copy(av(b4e, qoff, [[64, KC], [8, 4], [1, 4]]),
                                  av(D32, soff, [[32, KC], [1, 4], [4, 4]]))
                else:
                    v.tensor_scalar_mul(av(b4e, qoff, [[64, KC], [8, 4], [1, 4]]),
                                        av(D32, soff, [[32, KC], [1, 4], [4, 4]]), -1.0)

            # E = T' @ D: A=a3ve, B4=b4e; d1=g,d2=r2,j=r
            for kk in range(KC):
                cmat(a3ve, (k0 + kk) * 128, 8, 4, b4e, kk * 64, 32, 8, 4,
                     Ng, Nrf, Nrf, p_off=kk * 1024)
            kt, ko = fold_reduce(Pt, 0, KC * 2 * Ng * Nrf, 8)
            v.tensor_copy(ke[:, 0:KC * 128], av(kt, ko, [[1, KC * 128]]))

            # W = Lam @ V: A=lam [g,c,g'], B4=b4cw; d1=g,d2=r2,j=g'
            for kk in range(KC):
                cmat(lam_t, (k0 + kk) * WLAM, 32, 16, b4cw, kk * 256, 128, 32, 16,
                     Ng, Nrf, Ng, p_off=kk * 4096)
            kt, ko = fold_reduce(Pt, 0, KC * 2 * Ng * Nrf, 32)
            # mgf = W - E into mgf_t, realified-signed per k [g,cc,r,oc]:
            # (0,0)=(1,1)=mre=KW0-KE0; (0,1)=mim=KW1-KE1; (1,0)=-mim
            for kk in range(KC):
                moff = (k0 + kk) * 256
                kbw = ko + kk * 128
                kbe = kk * 128
                v.tensor_sub(av(mgf_t, moff, [[9, 2], [16, 16], [2, 4]]),
                             av(kt, kbw, [[0, 2], [4, 16], [1, 4]]),
                             av(ke, kbe, [[0, 2], [4, 16], [1, 4]]))
                v.tensor_sub(av(mgf_t, moff + 1, [[16, 16], [2, 4]]),
                             av(kt, kbw + 64, [[4, 16], [1, 4]]),
                             av(ke, kbe + 64, [[4, 16], [1, 4]]))
                v.tensor_sub(av(mgf_t, moff + 8, [[16, 16], [2, 4]]),
                             av(ke, kbe + 64, [[4, 16], [1, 4]]),
                             av(kt, kbw + 64, [[4, 16], [1, 4]]))

        # ---------- PE: eg = sum_k Gh @ mgf ----------
        # mgf_t -> mstg (k-major), then gather stationary sta:
        # rows 32*kq+2g+cc (4 k per matmul), cols (sample*4+kgrp)*8+roc
        nc.sync.dma_start(out=mstg[h].rearrange("k p w -> p k w"),
                          in_=mgf_t.rearrange("p (k w) -> p k w", k=Nk))
        sta = blk.tile([P, 4096], F16, name="sta")
        m0 = mstg[h, 0]
        for kq in range(4):
            for kgrp in range(4):
                nc.sync.dma_start(
                    out=bass.AP(tensor=sta.tensor,
                                offset=sta.offset + 32 * kq * 4096 + kgrp * 8,
                                ap=[[4096, 32], [32, P], [1, 8]]),
                    in_=bass.AP(tensor=m0.tensor,
                                offset=m0.offset + (kgrp * 4 + kq) * P * 256,
                                ap=[[8, 32], [256, P], [1, 8]]))
        # moving: separate tile so next block's GF can overlap this tail
        g3mov = blk.tile([P, 64 * Nk * Ng], F16, name="g3mov")
        nc.sync.dma_start(out=g3mov, in_=g3[h])
        egsb = blk.tile([P, 1024], F32, name="egsb")
        for sgrp in range(32):
            pt = psum.tile([P, 32], F32, name="peg")
            for sq in range(4):
                s = sgrp * 4 + sq
                for kgrp in range(4):
                    nc.tensor.matmul(
                        out=pt[32 * sq:32 * sq + 8, :],
                        lhsT=av(sta, (s * 4 + kgrp) * 8, [[1, 8]]),
                        rhs=av(g3mov, (s * 4 + kgrp) * 32, [[1, 32]]),
                        start=(kgrp == 0), stop=(kgrp == 3),
                        tile_position=(0, 32 * sq))
            nc.scalar.activation(egsb[:, sgrp * 32:(sgrp + 1) * 32], pt, AF.Copy)
        nc.sync.dma_start(out=estg[h], in_=egsb)
        # gather eg to sample-major egT [p, (c,n,r)]: per (oc, s4)
        e0 = estg[h]
        for oc in (0, 1):
            for s4 in range(4):
                nc.sync.dma_start(
                    out=bass.AP(tensor=egT.tensor,
                                offset=egT.offset + s4 * 256 + oc * 128,
                                ap=[[1024, 32], [4, 32], [1, 4]]),
                    in_=bass.AP(tensor=e0.tensor,
                                offset=e0.offset + (32 * s4 + oc) * 1024,
                                ap=[[32, 32], [1, 32], [2048, 4]]))

        # ---------- epilogue ----------
        if dbg is not None:
            nc.sync.dma_start(out=dbg["egT"][hs:hs + P], in_=egT)
        egr = egT[:, 0:128]
        egi = egT[:, 128:256]
        v.tensor_scalar_mul(egr, egr, -1.0 / Nk)
        v.tensor_scalar_mul(egi, egi, -1.0 / Nk)
        v.tensor_mul(egr, egr, pm_t)
        v.tensor_mul(egi, egi, pm_t)

        # step MLP (fixed leaky-relu)
        iq = blk.tile([P, 64], F32, name="iq")
        v.tensor_reduce(av(iq, 0, [[1, 32], [0, 1]]),
                        av(f32_t, 0, [[4, 32], [1, 4]]), axis=AX, op=ADD)
        v.tensor_reduce(av(iq, 32, [[1, 32], [0, 1]]),
                        av(f32_t, 128, [[4, 32], [1, 4]]), axis=AX, op=ADD)
        v.tensor_mul(iq, iq, bnsc)
        v.tensor_add(iq, iq, bnsh)
        v.tensor_mul(iq, iq, dw_t)
        z = blk.tile([P, 1], F32, name="z")
        v.tensor_reduce(z, iq.unsqueeze(1), axis=AX, op=ADD)
        v.tensor_add(z, z, db_t)
        smax = blk.tile([P, 1], F32, name="smax")
        step = blk.tile([P, 1], F32, name="step")
        v.tensor_scalar_max(smax, z, 0.0)
        v.tensor_scalar_min(step, z, 0.0)
        v.scalar_tensor_tensor(out=step, in0=step, scalar=ALPHA, in1=smax,
                               op0=MULT, op1=ADD)

        # proj = Re(eg * conj(Frf)); rg = eg - proj*Frf
        proj = blk.tile([P, 128], F32, name="proj")
        t0 = sc1[:, 0:128]
        t1 = sc2[:, 0:128]
        v.tensor_mul(proj, egr, fre)
        v.tensor_mul(t0, egi, fim)
        v.tensor_add(proj, proj, t0)
        rgr = blk.tile([P, 128], F32, name="rgr")
        rgi = blk.tile([P, 128], F32, name="rgi")
        v.tensor_mul(t0, proj, fre)
        v.tensor_sub(rgr, egr, t0)
        v.tensor_mul(t0, proj, fim)
        v.tensor_sub(rgi, egi, t0)

        # nrm; sc = -step/nrm
        n2 = blk.tile([P, 1], F32, name="n2")
        v.tensor_mul(t0, rgr, rgr)
        v.tensor_mul(t1, rgi, rgi)
        v.tensor_add(t0, t0, t1)
        v.tensor_reduce(n2, t0.unsqueeze(1), axis=AX, op=ADD)
        nc.scalar.activation(n2, n2, AF.Sqrt, bias=zero1)
        v.reciprocal(n2, n2)
        v.tensor_mul(n2, n2, step)
        v.tensor_scalar_mul(n2, n2, -1.0)
        fnr = blk.tile([P, 128], F32, name="fnr")
        fni = blk.tile([P, 128], F32, name="fni")
        v.scalar_tensor_tensor(out=fnr, in0=rgr, scalar=n2, in1=fre,
                               op0=MULT, op1=ADD)
        v.scalar_tensor_tensor(out=fni, in0=rgi, scalar=n2, in1=fim,
                               op0=MULT, op1=ADD)

        # scale = relu(|fnew|-1)+1 ; out = fnew/scale (interleaved)
        m2 = blk.tile([P, 128], F32, name="m2")
        v.tensor_mul(m2, fnr, fnr)
        v.tensor_mul(t0, fni, fni)
        v.tensor_add(m2, m2, t0)
        nc.scalar.activation(m2, m2, AF.Sqrt, bias=zero1)
        nc.scalar.activation(m2, m2, AF.Relu, bias=neg1)
        v.tensor_scalar_add(m2, m2, 1.0)
        v.reciprocal(m2, m2)
        ob = blk.tile([P, 256], F32, name="ob")
        v.tensor_mul(av(ob, 0, [[2, 128]]), fnr, m2)
        v.tensor_mul(av(ob, 1, [[2, 128]]), fni, m2)
        nc.sync.dma_start(out=out_v[hs:hs + P], in_=ob)


def inv4_batched(v, scr, S32, Y32):
    """Y32 = inv(S32) for Nk batched 4x4 complex mats, layout [k,c,i,j] f32.
    2x2 block Schur: S=[[A,B],[C,D]] -> iA, E=D-C iA B, iE, assemble."""
    KS = 32  # per-k stride

    # mini complex 2x2 tiles: layout [k, c(4), i(2), j(1)] width Nk*8
    mt = {nm: scr.tile([P, Nk * 8], F32, name="m_" + nm)
          for nm in ("iA", "iE", "P2", "Q2", "T2", "E2")}
    sc = scr.tile([P, Nk * 16], F32, name="m_sc")

    def sl(i):  # scratch slot [p, Nk]
        return av(sc, i * Nk, [[1, Nk]])

    def ent(t, base, c, i, j, mini=False):
        if mini:
            return av(t, c * 4 + i * 2 + j, [[8, Nk]])
        return av(t, base + c * 16 + i * 4 + j, [[KS, Nk]])

    def blkv(t, base, c, mini=False):
        if mini:
            return av(t, c * 4, [[8, Nk], [2, 2], [1, 2]])
        return av(t, base + c * 16, [[KS, Nk], [4, 2], [1, 2]])

    def c2inv(dst, src, sbase, smini):
        """dst (mini) = inv of 2x2 complex block of src at sbase."""
        e = lambda c, i, j: ent(src, sbase, c, i, j, smini)
        dre, dim, q0, q1 = sl(0), sl(1), sl(2), sl(3)
        # det = s00*s11 - s01*s10
        v.tensor_mul(dre, e(0, 0, 0), e(0, 1, 1))
        v.tensor_mul(q0, e(1, 0, 0), e(1, 1, 1))
        v.tensor_sub(dre, dre, q0)
        v.tensor_mul(q0, e(0, 0, 1), e(0, 1, 0))
        v.tensor_sub(dre, dre, q0)
        v.tensor_mul(q0, e(1, 0, 1), e(1, 1, 0))
        v.tensor_add(dre, dre, q0)
        v.tensor_mul(dim, e(0, 0, 0), e(1, 1, 1))
        v.tensor_mul(q0, e(1, 0, 0), e(0, 1, 1))
        v.tensor_add(dim, dim, q0)
        v.tensor_mul(q0, e(0, 0, 1), e(1, 1, 0))
        v.tensor_sub(dim, dim, q0)
        v.tensor_mul(q0, e(1, 0, 1), e(0, 1, 0))
        v.tensor_sub(dim, dim, q0)
        # inv det (conj form)
        v.tensor_mul(q0, dre, dre)
        v.tensor_mul(q1, dim, dim)
        v.tensor_add(q0, q0, q1)
        v.reciprocal(q0, q0)
        v.tensor_mul(dre, dre, q0)
        v.tensor_mul(dim, dim, q0)
        v.tensor_scalar_mul(dim, dim, -1.0)
        # adj entries into sc slots 4..11: [a11,-a01,-a10,a00] per comp
        for c in (0, 1):
            base_sl = 4 + c * 4
            v.tensor_copy(sl(base_sl + 0), e(c, 1, 1))
            v.tensor_scalar_mul(sl(base_sl + 1), e(c, 0, 1), -1.0)
            v.tensor_scalar_mul(sl(base_sl + 2), e(c, 1, 0), -1.0)
            v.tensor_copy(sl(base_sl + 3), e(c, 0, 0))
        adjr = av(sc, 4 * Nk, [[1, Nk], [2 * Nk, 2], [Nk, 2]])
        adji = av(sc, 8 * Nk, [[1, Nk], [2 * Nk, 2], [Nk, 2]])
        q2 = av(sc, 12 * Nk, [[1, Nk], [2 * Nk, 2], [Nk, 2]])
        dreb = av(sc, 0, [[1, Nk], [0, 2], [0, 2]])
        dimb = av(sc, Nk, [[1, Nk], [0, 2], [0, 2]])
        dr = blkv(dst, 0, 0, True)
        di = blkv(dst, 0, 1, True)
        # re = adjr*dre - adji*dim ; im = adjr*dim + adji*dre
        v.tensor_mul(q2, adjr, dreb)
        v.tensor_mul(dr, adji, dimb)
        v.tensor_sub(dr, q2, dr)
        v.tensor_mul(q2, adjr, dimb)
        v.tensor_mul(di, adji, dreb)
        v.tensor_add(di, q2, di)

    def c2mul(dst, x, xbase, xmini, y, ybase, ymini,
              sub_from=None, sf_base=0, sf_mini=False, negate=False):
        """dst (mini) = x@y | sub_from - x@y | -(x@y), 2x2 complex."""
        t0f = av(sc, 0, [[1, Nk], [Nk, 8]])
        t1f = av(sc, 8 * Nk, [[1, Nk], [Nk, 8]])

        def bxr(t, base, c, i, mini):
            # row i of x block: [k, j(bcast), m]
            if mini:
                return av(t, c * 4 + i * 2, [[8, Nk], [0, 2], [1, 2]])
            return av(t, base + c * 16 + i * 4, [[KS, Nk], [0, 2], [1, 2]])

        def by(t, base, c, mini):
            # y block as [k, j, m] with entry (m,j)
            if mini:
                return av(t, c * 4, [[8, Nk], [1, 2], [2, 2]])
            return av(t, base + c * 16, [[KS, Nk], [1, 2], [4, 2]])

        for comp in (0, 1):
            for i in (0, 1):
                t0r = av(sc, i * 4 * Nk, [[1, Nk], [2 * Nk, 2], [Nk, 2]])
                t1r = av(sc, (8 + i * 4) * Nk, [[1, Nk], [2 * Nk, 2], [Nk, 2]])
                v.tensor_mul(t0r, bxr(x, xbase, 0, i, xmini), by(y, ybase, comp, ymini))
                v.tensor_mul(t1r, bxr(x, xbase, 1, i, xmini), by(y, ybase, 1 - comp, ymini))
            if comp == 0:
                v.tensor_sub(t0f, t0f, t1f)
            else:
                v.tensor_add(t0f, t0f, t1f)
            d = blkv(dst, 0, comp, True)
            red = av(sc, 8 * Nk, [[1, Nk], [Nk, 4], [0, 1]])
            v.tensor_reduce(red, av(sc, 0, [[1, Nk], [2 * Nk, 4], [Nk, 2]]),
                            axis=AX, op=ADD)
            redv = av(sc, 8 * Nk, [[1, Nk], [2 * Nk, 2], [Nk, 2]])
            if sub_from is not None:
                v.tensor_sub(d, blkv(sub_from, sf_base, comp, sf_mini), redv)
            elif negate:
                v.tensor_scalar_mul(d, redv, -1.0)
            else:
                v.tensor_copy(d, redv)

    iA, iE, P2, Q2, T2, E2 = (mt[n] for n in ("iA", "iE", "P2", "Q2", "T2", "E2"))
    # S blocks: A off 0, B off 2, C off 8, D off 10
    c2inv(iA, S32, 0, False)
    c2mul(P2, iA, 0, True, S32, 2, False)                 # P2 = iA@B
    c2mul(Q2, S32, 8, False, iA, 0, True)                 # Q2 = C@iA
    c2mul(E2, S32, 8, False, P2, 0, True,
          sub_from=S32, sf_base=10, sf_mini=False)        # E2 = D - C@P2
    c2inv(iE, E2, 0, True)
    c2mul(T2, P2, 0, True, iE, 0, True, negate=True)      # T2 = -P2@iE = Y12
    # write Y12 -> Y32[0:2,2:4]
    for c in (0, 1):
        v.tensor_copy(blkv(Y32, 2, c), blkv(T2, 0, c, True))
    c2mul(E2, iE, 0, True, Q2, 0, True, negate=True)      # E2 = -iE@Q2 = Y21
    for c in (0, 1):
        v.tensor_copy(blkv(Y32, 8, c), blkv(E2, 0, c, True))
    # Y11 = iA - Y12@Q2  (T2 holds Y12; write to E2 to avoid dst/src alias)
    c2mul(E2, T2, 0, True, Q2, 0, True,
          sub_from=iA, sf_base=0, sf_mini=True)
    for c in (0, 1):
        v.tensor_copy(blkv(Y32, 0, c), blkv(E2, 0, c, True))
    for c in (0, 1):
        v.tensor_copy(blkv(Y32, 10, c), blkv(iE, 0, c, True))


# ================= host side =================

_NC_CACHE = {}


def _prep(inputs):
    f16, f32 = np.float16, np.float32
    Gr = np.asarray(inputs["G_re"], f32)
    Gi = np.asarray(inputs["G_im"], f32)
    Lr = np.asarray(inputs["Lam_re"], f32)
    Li = np.asarray(inputs["Lam_im"], f32)
    fr = np.asarray(inputs["Frf_re"], f32)
    fi = np.asarray(inputs["Frf_im"], f32)


    lamx = np.empty((Nk, B, Ng, 2, Ng), f16)    # Lam: [g,c,g']
    lamx[..., 0, :] = Lr
    lamx[..., 1, :] = Li
    frT = fr.transpose(0, 2, 1)                  # [B,r,n]
    fiT = fi.transpose(0, 2, 1)
    fb4 = np.empty((B, 2, Nrf, 2, Nt), f16)     # B4(Frf): [oc,r,c,n]
    fb4[:, 0, :, 0, :] = frT
    fb4[:, 0, :, 1, :] = -fiT
    fb4[:, 1, :, 0, :] = fiT
    fb4[:, 1, :, 1, :] = frT
    f32x = np.empty((B, 2, Nt, Nrf), f32)
    f32x[:, 0] = fr
    f32x[:, 1] = fi
    u32 = np.ascontiguousarray(
        (1.0 / np.asarray(inputs["Beta_re"], f32).reshape(Nk, B).T))
    aux = np.concatenate([
        np.asarray(inputs["bn_gamma"], f32).ravel(),
        np.asarray(inputs["bn_beta"], f32).ravel(),
        np.asarray(inputs["bn_mean"], f32).ravel(),
        np.asarray(inputs["bn_var"], f32).ravel(),
        np.asarray(inputs["dense_w"], f32).ravel(),
        np.asarray(inputs["dense_b"], f32).ravel(),
        np.asarray(inputs["P_mask"], f32).ravel()])

    in_maps = []
    for c in range(NCORES):
        s = slice(c * BL, (c + 1) * BL)
        # gpe: moving G for PE-GF: [h, row=64s+32cc+n, pair*256 + k*16+g]
        Xc = np.stack([Gr[:, s], Gi[:, s]])          # [cc,k,BL,n,g]
        Xc = Xc.reshape(2, Nk, H, 64, 2, Nt, Ng)     # [cc,k,h,q,ss,n,g]
        gpe = np.ascontiguousarray(
            Xc.transpose(2, 4, 0, 5, 3, 1, 6).reshape(H, P, 64 * Nk * Ng)
        ).astype(f16)
        # fpe: block-diag stationary: rows as gpe, cols pair*16 + 8s+2r+oc
        # oc=0 (re): (cc0: fr, cc1: fi) ; oc=1 (-im, conj): (cc0: -fi, cc1: fr)
        frc = fr[s].reshape(H, 64, 2, Nt, Nrf)       # [h,q,ss,n,r]
        fic = fi[s].reshape(H, 64, 2, Nt, Nrf)
        fpe_a = np.zeros((H, 2, 2, Nt, 64, 16), np.float32)  # [h,ss,cc,n,q,col]
        for ss in (0, 1):
            fq = frc[:, :, ss].transpose(0, 2, 1, 3)  # [h,n,q,r]
            gq = fic[:, :, ss].transpose(0, 2, 1, 3)
            cb = np.empty((2, H, Nt, 64, Nrf, 2), np.float32)  # [cc,h,n,q,r,oc]
            cb[0, ..., 0] = fq
            cb[0, ..., 1] = -gq
            cb[1, ..., 0] = gq
            cb[1, ..., 1] = fq
            for cc in (0, 1):
                fpe_a[:, ss, cc, :, :, 8 * ss:8 * ss + 8] = \
                    cb[cc].reshape(H, Nt, 64, 8)
        fpe = np.ascontiguousarray(fpe_a.reshape(H, P, 64 * 16)).astype(f16)
        # g3: eg moving operand: rows 32*kq+2g+cc, cols (sample*4+kgrp)*32+n
        Yc = np.stack([Gr[:, s], Gi[:, s]])          # [cc,k,BL,n,g]
        Yc = Yc.reshape(2, 4, 4, H, P, Nt, Ng)       # [cc,kgrp,kq,h,smp,n,g]
        g3 = np.ascontiguousarray(
            Yc.transpose(3, 2, 6, 0, 4, 1, 5).reshape(H, P, 32 * Nk * Nt)
        ).astype(f16)
        in_maps.append({
            "gpe": gpe,
            "fpe": fpe,
            "g3": g3,
            "lam": np.ascontiguousarray(lamx[:, s]).reshape(Nk, BL, WLAM),
            "fb4": np.ascontiguousarray(fb4[s]).reshape(BL, 512),
            "f32t": np.ascontiguousarray(f32x[s]).reshape(BL, 256),
            "u32": np.ascontiguousarray(u32[s]),
            "aux": aux,
        })
    return in_maps


def kernel(**inputs):
    in_maps = _prep(inputs)
    if "nc" not in _NC_CACHE:
        _NC_CACHE["nc"] = build_nc()
    nc = _NC_CACHE["nc"]
    from concourse.bass_utils import run_bass_kernel_spmd
    res = run_bass_kernel_spmd(nc, in_maps, core_ids=list(range(NCORES)))
    outs = [res.results[c]["out"] for c in range(NCORES)]
    full = np.concatenate(outs, axis=0).reshape(B, Nt, Nrf, 2)
    return np.ascontiguousarray(full).view(np.complex64).reshape(B, Nt, Nrf)


def kernel_profiled(**inputs):
    """Returns modeled HW exec time (ns) from the cost-model timeline sim."""
    if "nc" not in _NC_CACHE:
        _NC_CACHE["nc"] = build_nc()
    from concourse.timeline_sim import TimelineSim
    ts = TimelineSim(_NC_CACHE["nc"], no_exec=True)
    return int(ts.simulate())


# revision 56
# speedup vs baseline: 1.0568x; 1.0568x over previous
"""Trainium2 Bass kernel for nn_DUP_block_90391881712206.

Math per (k,b) via Woodbury (no 16x16 inversions):
    GF = conj(Gh)^T @ Frf            [Ng,Nrf]
    T' = Lam @ GF
    S  = I + u * GF^H @ T'  (u = 1/beta)
    Y  = inv(S)  (4x4 complex, 2x2 block Schur, f32)
    V  = u * T' @ Y
    C  = T'^H @ V ; D = u * Y @ C
    E  = T' @ D ; W = Lam @ V ; mgf = W - E
    eg_k = Gh @ mgf
eg = -mean_k(eg_k) * P_mask, then Riemannian step + unit-disk clamp.

Implementation: data-parallel over B across 8 cores (256 b/core, 2 blocks
of 128 partitions). Batched small complex matmuls on DVE in fp16: one
broadcast-AP multiply builds the product tensor [outc,d1,d2,c,j] (packed
last dim -> 2x DVE mode), then log2 halving-fold adds reduce segments
(packed -> 2x). Host pre-lays inputs (conj baked into G's imag for GF).
"""

import numpy as np
import sys
from contextlib import ExitStack

sys.path.insert(0, "/opt/trn_rl_repo")

import concourse.bass as bass
import concourse.bacc as bacc_mod
import concourse.tile as tile
from concourse import mybir

Nk, B, Nt, Nrf, Ng = 16, 2048, 32, 4, 16
NCORES = 8
BL = B // NCORES
P = 128
H = BL // P
KC = 2                     # k per chunk
NCH = Nk // KC
BN_EPS = 1e-3
ALPHA = 0.1

F16 = mybir.dt.float16
F32 = mybir.dt.float32
AX = mybir.AxisListType.X
ADD = mybir.AluOpType.add
SUB = mybir.AluOpType.subtract
MULT = mybir.AluOpType.mult
AF = mybir.ActivationFunctionType

# per-k widths (fp16 elems)
WG1 = Ng * 2 * Nt          # 1024  g1 [g,c,n]
WG2 = Nt * 2 * Ng          # 1024  g2 [n,c,g]
WLAM = Ng * 3 * Ng         # 768   lam 3-slot [g,s,g'] (Karatsuba)


def av(t, off, dims):
    """AP view of tile t at free-offset `off` with free dims [[stride,n],..]."""
    return bass.AP(tensor=t.tensor, offset=t.offset + off,
                   ap=[list(t.ap[0])] + [list(d) for d in dims])


def build_nc(debug=False):
    nc = bacc_mod.Bacc()
    gpe = nc.dram_tensor("gpe", [H, P, 64 * Nk * Ng], F16, kind="ExternalInput")
    fpe = nc.dram_tensor("fpe", [H, P, 64 * 16], F16, kind="ExternalInput")
    stg = nc.dram_tensor("stg", [H, 16, P, 256], F16, kind="Internal")
    g3 = nc.dram_tensor("g3", [H, P, 32 * Nk * Nt], F16, kind="ExternalInput")
    mstg = nc.dram_tensor("mstg", [H, Nk, P, 256], F16, kind="Internal")
    estg = nc.dram_tensor("estg", [H, P, 1024], F32, kind="Internal")
    lam = nc.dram_tensor("lam", [Nk, BL, WLAM], F16, kind="ExternalInput")
    fb4 = nc.dram_tensor("fb4", [BL, 2 * Nrf * 2 * Nt], F16, kind="ExternalInput")
    f32t = nc.dram_tensor("f32t", [BL, 2 * Nt * Nrf], F32, kind="ExternalInput")
    u32 = nc.dram_tensor("u32", [BL, Nk], F32, kind="ExternalInput")
    aux = nc.dram_tensor("aux", [5 * 2 * Nt + 1 + Nt * Nrf], F32, kind="ExternalInput")
    out = nc.dram_tensor("out", [BL, Nt * Nrf * 2], F32, kind="ExternalOutput")
    dbg = None
    if debug:
        dbg = {nm: nc.dram_tensor("dbg_" + nm, [BL, w], dt, kind="ExternalOutput")
               for nm, w, dt in (("a3s", Nk * 128, F16), ("a3ve", Nk * 128, F16),
                                 ("S32", Nk * 32, F32), ("Y32", Nk * 32, F32),
                                 ("vw", Nk * 256, F16), ("keg", Nk * 256, F16),
                                 ("egT", 256, F32))}

    with ExitStack() as ctx:
        tc = ctx.enter_context(tile.TileContext(nc))
        kern(ctx, tc, gpe, fpe, stg, g3, mstg, estg, lam, fb4, f32t, u32, aux,
             out, dbg)
    if not nc.is_finalized():
        nc.finalize()
    return nc


def kern(ctx, tc, gpe, fpe, stg, g3, mstg, estg, lam, fb4, f32t, u32, aux,
         out, dbg=None):
    nc = tc.nc
    v = nc.vector

    singles = ctx.enter_context(tc.tile_pool(name="singles", bufs=1))
    loads = ctx.enter_context(tc.tile_pool(name="loads", bufs=3))
    blk = ctx.enter_context(tc.tile_pool(name="blk", bufs=1))
    scr = ctx.enter_context(tc.tile_pool(name="scr", bufs=1))
    psum = ctx.enter_context(tc.tile_pool(name="psum", bufs=4, space="PSUM"))

    # ---- broadcast-load aux params ----
    aux_t = singles.tile([P, 449], F32, name="aux_bc")
    aux_ap = aux[:]
    src = bass.AP(tensor=aux_ap.tensor, offset=aux_ap.offset,
                  ap=[[0, P]] + list(aux_ap.ap))
    nc.gpsimd.dma_start(out=aux_t, in_=src)
    gam_t, bb_t = aux_t[:, 0:64], aux_t[:, 64:128]
    bm_t, bv_t = aux_t[:, 128:192], aux_t[:, 192:256]
    dw_t, db_t = aux_t[:, 256:320], aux_t[:, 320:321]
    pm_t = aux_t[:, 321:449]

    zero1 = singles.tile([P, 1], F32, name="zero1")
    neg1 = singles.tile([P, 1], F32, name="neg1")
    v.memset(zero1, 0.0)
    v.memset(neg1, -1.0)
    eye16 = singles.tile([P, 16], F32, name="eye16")
    v.memset(eye16, 0.0)
    v.memset(av(eye16, 0, [[5, 4]]), 1.0)

    # bn scale/shift
    bnsc = singles.tile([P, 64], F32, name="bnsc")
    bnsh = singles.tile([P, 64], F32, name="bnsh")
    v.tensor_scalar_add(bnsc, bv_t, BN_EPS)
    v.reciprocal(bnsc, bnsc)
    nc.scalar.activation(bnsc, bnsc, AF.Sqrt, bias=zero1)
    v.tensor_mul(bnsc, bnsc, gam_t)
    v.tensor_mul(bnsh, bm_t, bnsc)
    v.tensor_sub(bnsh, bb_t, bnsh)

    # ---- scratch tiles ----
    # product tensors + fold ping-pong (sized for biggest cmat: 4w=8192/k)
    Pt = scr.tile([P, KC * 4096], F16, name="Pt")
    Pf1 = scr.tile([P, KC * 2048], F16, name="Pf1")
    Pf2 = scr.tile([P, KC * 1024], F16, name="Pf2")
    sc0 = scr.tile([P, KC * 256], F32, name="sc0")   # f32 scratch (D etc)
    sc1 = scr.tile([P, 256], F32, name="sc1")
    sc2 = scr.tile([P, 256], F32, name="sc2")

    def fold_reduce(src_t, src_off, nseg, L, eng1=None):
        """Sum contiguous segments of length L (pow2) via halving adds.
        Returns (tile, offset) of compact [nseg] result. eng1: engine for
        the first (widest) fold level (e.g. nc.gpsimd to offload)."""
        cur_t, cur_off, cl = src_t, src_off, L
        ping = [Pf1, Pf2]
        pi = 0
        first = True
        while cl > 1:
            half = cl // 2
            dst = ping[pi]
            pi ^= 1
            eng = eng1 if (first and eng1 is not None) else v
            eng.tensor_add(av(dst, 0, [[half, nseg], [1, half]]),
                           av(cur_t, cur_off, [[cl, nseg], [1, half]]),
                           av(cur_t, cur_off + half, [[cl, nseg], [1, half]]))
            cur_t, cur_off, cl = dst, 0, half
            first = False
        return cur_t, cur_off

    def cmat(a_t, a_off, a_d1s, a_cs, b4_t, b4_off, b4_ocs, b4_d2s, b4_cs,
             d1, d2, j, p_off=0):
        """Product P[outc,d1,d2,c,j] = A[d1,c,j] * B4[outc,d2,c,j], then
        fold-reduce (c,j) -> K [outc,d1,d2] compact fp16. One k at a time
        for the mul (4-dim APs); returns nothing (call fold separately)."""
        L = 2 * j
        w2 = d1 * d2 * L
        a_v = av(a_t, a_off, [[a_d1s, d1], [0, d2], [1, L]])
        for oc in range(2):
            ov = av(Pt, p_off + oc * w2, [[d2 * L, d1], [L, d2], [1, L]])
            b_v = av(b4_t, b4_off + oc * b4_ocs, [[0, d1], [b4_d2s, d2], [1, L]])
            v.tensor_mul(ov, a_v, b_v)

    def cmat3(a_t, a_off, a_d1s, a_ss, b3_t, b3_off, b3_ss, b3_d2s,
              d1, d2, j, p_off=0):
        """Gauss 3-mult: P3[s,d1,d2,j] = A3[d1,s,j] * B3[s,d2,j];
        fold j -> K [s,d1,d2]; re=K1-K3, im=K1+K2."""
        w1 = d1 * d2 * j
        for sl in range(3):
            ov = av(Pt, p_off + sl * w1, [[d2 * j, d1], [j, d2], [1, j]])
            a_v = av(a_t, a_off + sl * a_ss, [[a_d1s, d1], [0, d2], [1, j]])
            b_v = av(b3_t, b3_off + sl * b3_ss, [[0, d1], [b3_d2s, d2], [1, j]])
            v.tensor_mul(ov, a_v, b_v)

    # ============ per-block main ============
    out_v = out
    for h in range(H):
        hs = h * P

        fb4_t = blk.tile([P, 512], F16, name="fb4_t")
        f32_t = blk.tile([P, 256], F32, name="f32_t")
        u_t = blk.tile([P, Nk], F32, name="u_t")
        nc.sync.dma_start(out=fb4_t, in_=fb4[hs:hs + P])
        nc.sync.dma_start(out=f32_t, in_=f32t[hs:hs + P])
        nc.sync.dma_start(out=u_t, in_=u32[hs:hs + P])
        fre = f32_t[:, 0:128]
        fim = f32_t[:, 128:256]

        lam_t = blk.tile([P, Nk * WLAM], F16, name="lam_t")
        nc.sync.dma_start(out=lam_t.rearrange("p (k w) -> p k w", k=Nk),
                          in_=lam[:, hs:hs + P].rearrange("k b w -> b k w"))

        # all-k mid tensors (fp16 per-k layouts)
        a3ve = blk.tile([P, Nk * 128], F16, name="a3ve")  # T' [g,c,r]
        a3c = blk.tile([P, Nk * 128], F16, name="a3c")    # conj(T') [r,c,g']
        b4s = blk.tile([P, Nk * 256], F16, name="b4s")    # B4(T') [oc,r2,c,g']
        a3s = blk.tile([P, Nk * 128], F16, name="a3s")    # conj(GF) [r,c,g']
        b3t = blk.tile([P, Nk * 192], F16, name="b3t")    # B3(GF) [s,r,g']
        b3w = blk.tile([P, KC * 192], F16, name="b3w")    # B3(V) [s,r2,g']
        S32 = blk.tile([P, Nk * 32], F32, name="S32")     # [k,c,i,j]
        Y32 = blk.tile([P, Nk * 32], F32, name="Y32")
        Yu32 = blk.tile([P, Nk * 32], F32, name="Yu32")
        b4y = blk.tile([P, Nk * 64], F16, name="b4y")     # B4(Yu) [oc,r2,c,r]
        b4cw = blk.tile([P, KC * 256], F16, name="b4cw")  # B4(V) [oc,r2,c,g']
        C32 = blk.tile([P, KC * 32], F32, name="C32")
        D32 = blk.tile([P, KC * 32], F32, name="D32")
        b4e = blk.tile([P, KC * 64], F16, name="b4e")     # B4(D) [oc,r2,c,r]
        ke = blk.tile([P, KC * 128], F16, name="ke")      # saved K of E
        egT = blk.tile([P, 256], F32, name="egT")
        mgf_t = blk.tile([P, Nk * 256], F16, name="mgf_t")

        # ---------- PE: GF for all k ----------
        # stationary fpe [128=(s,cc,n), 16=(s,r,oc)] per pair (block-diag Frf);
        # moving gmov [128, 256=(k,g)] per pair; out psum [16, 256] per pair.
        gmov = blk.tile([P, 64 * Nk * Ng], F16, name="gmov")
        fpe_t = blk.tile([P, 64 * 16], F16, name="fpe_t")
        nc.sync.dma_start(out=gmov, in_=gpe[h])
        nc.sync.dma_start(out=fpe_t, in_=fpe[h])
        for grp in range(16):
            pt = psum.tile([P, 256], F32, name="pgf")
            for qq in range(4):
                pair = grp * 4 + qq
                nc.tensor.matmul(out=pt[32 * qq:32 * qq + 16, :],
                                 lhsT=fpe_t[:, pair * 16:(pair + 1) * 16],
                                 rhs=av(gmov, pair * 256, [[1, 256]]),
                                 start=True, stop=True,
                                 tile_position=(0, 32 * qq))
            sgf = loads.tile([P, 256], F16, name="sgf")
            nc.scalar.activation(sgf, pt, AF.Copy)
            nc.sync.dma_start(out=stg[h, grp], in_=sgf)
        # stg row (per grp): 32*qq + 8s + 2r + oc ; col = k*16+g.
        # gather to a3s [p=sample, (r,c,k,g')]: 1 DMA per s
        st0 = stg[h, 0]
        pstride = Nk * 128
        for s in (0, 1):
            for r in range(4):
                for c in range(2):
                    nc.sync.dma_start(
                        out=bass.AP(tensor=a3s.tensor,
                                    offset=a3s.offset + s * pstride + r * 32 + c * 16,
                                    ap=[[2 * pstride, 64], [128, 16], [1, 16]]),
                        in_=bass.AP(tensor=st0.tensor,
                                    offset=st0.offset + s * 2048 + r * 512 + c * 256,
                                    ap=[[8192, 64], [16, 16], [1, 16]]))

        # b3t = B3(GF) slots (Gre, Gim-Gre, Gre+Gim); a3s = (Gre, -Gim)
        v.tensor_copy(av(b3t, 0, [[192, Nk], [16, 4], [1, 16]]),
                      av(a3s, 0, [[128, Nk], [32, 4], [1, 16]]))
        v.scalar_tensor_tensor(
            out=av(b3t, 64, [[192, Nk], [16, 4], [1, 16]]),
            in0=av(a3s, 16, [[128, Nk], [32, 4], [1, 16]]), scalar=-1.0,
            in1=av(a3s, 0, [[128, Nk], [32, 4], [1, 16]]),
            op0=MULT, op1=SUB)
        v.tensor_sub(av(b3t, 128, [[192, Nk], [16, 4], [1, 16]]),
                     av(a3s, 0, [[128, Nk], [32, 4], [1, 16]]),
                     av(a3s, 16, [[128, Nk], [32, 4], [1, 16]]))

        # ---------- pass 1: T', S per k-chunk ----------
        for ch in range(NCH):
            k0 = ch * KC

            # T' = Lam @ GF (Gauss): A3=lam [g,s,g'], B3=b3t; d1=g,d2=r,j=g'
            for kk in range(KC):
                cmat3(lam_t, (k0 + kk) * WLAM, 48, 16, b3t, (k0 + kk) * 192, 64, 16,
                      Ng, Nrf, Ng, p_off=kk * 3072)
            kt, ko = fold_reduce(Pt, 0, KC * 3 * Ng * Nrf, 16)
            # K [kk, s, g, r] strides 192,64,4,1 ; re=K1-K3, im=K1+K2
            kg0 = k0
            K1 = av(kt, ko, [[192, KC], [4, 16], [1, 4]])
            K2 = av(kt, ko + 64, [[192, KC], [4, 16], [1, 4]])
            K3 = av(kt, ko + 128, [[192, KC], [4, 16], [1, 4]])
            K1t = av(kt, ko, [[192, KC], [1, 4], [4, 16]])
            K2t = av(kt, ko + 64, [[192, KC], [1, 4], [4, 16]])
            K3t = av(kt, ko + 128, [[192, KC], [1, 4], [4, 16]])
            # a3ve = T' [g,c,r]
            v.tensor_sub(av(a3ve, kg0 * 128, [[128, KC], [8, 16], [1, 4]]), K1, K3)
            v.tensor_add(av(a3ve, kg0 * 128 + 4, [[128, KC], [8, 16], [1, 4]]), K1, K2)
            # a3c = conj(T') [r,c,g']
            v.tensor_sub(av(a3c, kg0 * 128, [[128, KC], [32, 4], [1, 16]]), K1t, K3t)
            a3c_im = av(a3c, kg0 * 128 + 16, [[128, KC], [32, 4], [1, 16]])
            v.tensor_add(a3c_im, K1t, K2t)
            v.tensor_scalar_mul(a3c_im, a3c_im, -1.0)
            # b4s = B4(T') [oc,r2,c,g']
            v.tensor_sub(av(b4s, kg0 * 256, [[256, KC], [32, 4], [1, 16]]), K1t, K3t)
            v.tensor_copy(av(b4s, kg0 * 256 + 16, [[256, KC], [32, 4], [1, 16]]),
                          a3c_im)
            v.tensor_add(av(b4s, kg0 * 256 + 128, [[256, KC], [32, 4], [1, 16]]), K1t, K2t)
            v.tensor_sub(av(b4s, kg0 * 256 + 144, [[256, KC], [32, 4], [1, 16]]), K1t, K3t)

            # S' = GF^H T': A=a3s [r1,c,g'], B4=b4s; d1=r1,d2=r2,j=g'
            for kk in range(KC):
                cmat(a3s, (k0 + kk) * 128, 32, 16, b4s, (k0 + kk) * 256, 128, 32, 16,
                     Nrf, Nrf, Ng, p_off=kk * 1024)
            kt, ko = fold_reduce(Pt, 0, KC * 2 * Nrf * Nrf, 32)
            # K [kk, outc, r1, r2] strides 32,16,4,1 -> S32 [k,c,i,j]
            v.tensor_copy(av(S32, k0 * 32, [[32, KC], [16, 2], [1, 16]]),
                          av(kt, ko, [[32, KC], [16, 2], [1, 16]]))
            # S = I + u*S'
            v.tensor_mul(av(S32, k0 * 32, [[32, KC], [1, 32]]),
                         av(S32, k0 * 32, [[32, KC], [1, 32]]),
                         av(u_t, k0, [[1, KC], [0, 32]]))
            v.tensor_add(av(S32, k0 * 32, [[32, KC], [1, 16]]),
                         av(S32, k0 * 32, [[32, KC], [1, 16]]),
                         av(eye16, 0, [[0, KC], [1, 16]]))

        if dbg is not None:
            nc.sync.dma_start(out=dbg["a3s"][hs:hs + P], in_=a3s)
            nc.sync.dma_start(out=dbg["a3ve"][hs:hs + P], in_=a3ve)
            nc.sync.dma_start(out=dbg["S32"][hs:hs + P], in_=S32)

        # ---------- pass 2: batched 4x4 inversion (2x2 Schur), f32 ----------
        inv4_batched(v, scr, S32, Y32)
        # Yu = u*Y
        v.tensor_mul(av(Yu32, 0, [[32, Nk], [1, 32]]),
                     av(Y32, 0, [[32, Nk], [1, 32]]),
                     av(u_t, 0, [[1, Nk], [0, 32]]))
        # b4y = B4(Yu) [oc,r2,c,r], B[r2,r]=Yu[r,r2] (transposed)
        for (qoff, soff, sgn) in ((0, 0, 1.0), (4, 16, -1.0),
                                  (32, 16, 1.0), (36, 0, 1.0)):
            if sgn > 0:
                v.tensor_copy(av(b4y, qoff, [[64, Nk], [8, 4], [1, 4]]),
                              av(Yu32, soff, [[32, Nk], [1, 4], [4, 4]]))
            else:
                v.tensor_scalar_mul(av(b4y, qoff, [[64, Nk], [8, 4], [1, 4]]),
                                    av(Yu32, soff, [[32, Nk], [1, 4], [4, 4]]), -1.0)

        if dbg is not None:
            nc.sync.dma_start(out=dbg["Y32"][hs:hs + P], in_=Y32)

        # ---------- pass 3: V, C, D, E, W, mgf, eg per k-chunk ----------
        for ch in range(NCH):
            k0 = ch * KC

            # V = T' @ Yu: A=a3ve [g,c,r], B4=b4y; d1=g,d2=r2,j=r
            for kk in range(KC):
                cmat(a3ve, (k0 + kk) * 128, 8, 4, b4y, (k0 + kk) * 64, 32, 8, 4,
                     Ng, Nrf, Nrf, p_off=kk * 1024)
            ktv, kov = fold_reduce(Pt, 0, KC * 2 * Ng * Nrf, 8)
            kt, ko = ktv, kov
            # K [kk,outc,g,r2] strides 128,64,4,1 -> b4cw [oc,r2,c,g'](V)
            for kk in range(KC):
                kb = ko + kk * 128
                v.tensor_copy(av(b4cw, kk * 256, [[32, 4], [1, 16]]),
                              av(kt, kb, [[1, 4], [4, 16]]))
                v.tensor_scalar_mul(av(b4cw, kk * 256 + 16, [[32, 4], [1, 16]]),
                                    av(kt, kb + 64, [[1, 4], [4, 16]]), -1.0)
                v.tensor_copy(av(b4cw, kk * 256 + 128, [[32, 4], [1, 16]]),
                              av(kt, kb + 64, [[1, 4], [4, 16]]))
                v.tensor_copy(av(b4cw, kk * 256 + 144, [[32, 4], [1, 16]]),
                              av(kt, kb, [[1, 4], [4, 16]]))

            # W = Lam @ V (Gauss): B3(V) slots (Vre,Vim-Vre,Vre+Vim) [s,r2,g']
            v.tensor_copy(av(b3w, 0, [[192, KC], [16, 4], [1, 16]]),
                          av(ktv, kov, [[128, KC], [1, 4], [4, 16]]))
            v.tensor_sub(av(b3w, 64, [[192, KC], [16, 4], [1, 16]]),
                         av(ktv, kov + 64, [[128, KC], [1, 4], [4, 16]]),
                         av(ktv, kov, [[128, KC], [1, 4], [4, 16]]))
            v.tensor_add(av(b3w, 128, [[192, KC], [16, 4], [1, 16]]),
                         av(ktv, kov, [[128, KC], [1, 4], [4, 16]]),
                         av(ktv, kov + 64, [[128, KC], [1, 4], [4, 16]]))

            if dbg is not None:
                nc.sync.dma_start(out=dbg["vw"][hs:hs + P, k0 * 256:(k0 + KC) * 256],
                                  in_=b4cw)

            # C = T'^H @ V: A=a3c [r1,c,g'], B4=b4cw; d1=r1,d2=r2,j=g'
            for kk in range(KC):
                cmat(a3c, (k0 + kk) * 128, 32, 16, b4cw, kk * 256, 128, 32, 16,
                     Nrf, Nrf, Ng, p_off=kk * 1024)
            kt, ko = fold_reduce(Pt, 0, KC * 2 * Nrf * Nrf, 32)
            v.tensor_copy(av(C32, 0, [[32, KC], [16, 2], [1, 16]]),
                          av(kt, ko, [[32, KC], [16, 2], [1, 16]]))

            # D = u * Y @ C (f32). product dims per k: [r1, r2, m]
            for comp in (0, 1):
                for kk in range(KC):
                    yv = lambda c: av(Y32, (k0 + kk) * 32 + c * 16,
                                      [[4, 4], [0, 4], [1, 4]])
                    cv = lambda c: av(C32, kk * 32 + c * 16,
                                      [[0, 4], [1, 4], [4, 4]])
                    t0 = av(sc0, kk * 64, [[16, 4], [4, 4], [1, 4]])
                    t1 = av(sc0, KC * 64 + kk * 64, [[16, 4], [4, 4], [1, 4]])
                    v.tensor_mul(t0, yv(0), cv(comp))
                    v.tensor_mul(t1, yv(1), cv(1 - comp))
                t0f = av(sc0, 0, [[1, KC * 64]])
                t1f = av(sc0, KC * 64, [[1, KC * 64]])
                if comp == 0:
                    v.tensor_sub(t0f, t0f, t1f)
                else:
                    v.tensor_add(t0f, t0f, t1f)
                v.tensor_reduce(av(D32, comp * 16, [[32, KC], [1, 16], [0, 1]]),
                                av(sc0, 0, [[64, KC], [4, 16], [1, 4]]),
                                axis=AX, op=ADD)
            v.tensor_mul(av(D32, 0, [[32, KC], [1, 32]]),
                         av(D32, 0, [[32, KC], [1, 32]]),
                         av(u_t, k0, [[1, KC], [0, 32]]))
            # b4e = B4(D) [oc,r2,c,r], B[r2,r]=D[r,r2]
            for (qoff, soff, sgn) in ((0, 0, 1.0), (4, 16, -1.0),
                                      (32, 16, 1.0), (36, 0, 1.0)):
                if sgn > 0:
                    v.tensor_copy(av(b4e, qoff, [[64, KC], [8, 4], [1, 4]]),
                                  av(D32, soff, [[32, KC], [1, 4], [4, 4]]))
                else:
                    v.tensor_scalar_mul(av(b4e, qoff, [[64, KC], [8, 4], [1, 4]]),
                                        av(D32, soff, [[32, KC], [1, 4], [4, 4]]), -1.0)

            # E = T' @ D: A=a3ve, B4=b4e; d1=g,d2=r2,j=r
            for kk in range(KC):
                cmat(a3ve, (k0 + kk) * 128, 8, 4, b4e, kk * 64, 32, 8, 4,
                     Ng, Nrf, Nrf, p_off=kk * 1024)
            kt, ko = fold_reduce(Pt, 0, KC * 2 * Ng * Nrf, 8)
            v.tensor_copy(ke[:, 0:KC * 128], av(kt, ko, [[1, KC * 128]]))

            # (B3(V) for W built right after V fold, above)
            for kk in range(KC):
                cmat3(lam_t, (k0 + kk) * WLAM, 48, 16, b3w, kk * 192, 64, 16,
                      Ng, Nrf, Ng, p_off=kk * 3072)
            kt, ko = fold_reduce(Pt, 0, KC * 3 * Ng * Nrf, 16)
            # mgf = W - E into mgf_t [g,cc,r,oc]: Wre=K1-K3, Wim=K1+K2
            # (0,0)=(1,1)=mre=Wre-KE0; (0,1)=mim=Wim-KE1; (1,0)=-mim
            for kk in range(KC):
                moff = (k0 + kk) * 256
                kbw = ko + kk * 192
                kbe = kk * 128
                d9 = av(mgf_t, moff, [[9, 2], [16, 16], [2, 4]])
                v.tensor_sub(d9, av(kt, kbw, [[0, 2], [4, 16], [1, 4]]),
                             av(kt, kbw + 128, [[0, 2], [4, 16], [1, 4]]))
                v.tensor_sub(d9, d9, av(ke, kbe, [[0, 2], [4, 16], [1, 4]]))
                d1v = av(mgf_t, moff + 1, [[16, 16], [2, 4]])
                v.tensor_add(d1v, av(kt, kbw, [[4, 16], [1, 4]]),
                             av(kt, kbw + 64, [[4, 16], [1, 4]]))
                v.tensor_sub(d1v, d1v, av(ke, kbe + 64, [[4, 16], [1, 4]]))
                d8 = av(mgf_t, moff + 8, [[16, 16], [2, 4]])
                v.tensor_sub(d8, av(ke, kbe + 64, [[4, 16], [1, 4]]),
                             av(kt, kbw, [[4, 16], [1, 4]]))
                v.tensor_sub(d8, d8, av(kt, kbw + 64, [[4, 16], [1, 4]]))

        # ---------- PE: eg = sum_k Gh @ mgf ----------
        # mgf_t -> mstg (k-major), then gather stationary sta:
        # rows 32*kq+2g+cc (4 k per matmul), cols (sample*4+kgrp)*8+roc
        nc.sync.dma_start(out=mstg[h].rearrange("k p w -> p k w"),
                          in_=mgf_t.rearrange("p (k w) -> p k w", k=Nk))
        sta = blk.tile([P, 4096], F16, name="sta")
        m0 = mstg[h, 0]
        for kq in range(4):
            for kgrp in range(4):
                nc.sync.dma_start(
                    out=bass.AP(tensor=sta.tensor,
                                offset=sta.offset + 32 * kq * 4096 + kgrp * 8,
                                ap=[[4096, 32], [32, P], [1, 8]]),
                    in_=bass.AP(tensor=m0.tensor,
                                offset=m0.offset + (kgrp * 4 + kq) * P * 256,
                                ap=[[8, 32], [256, P], [1, 8]]))
        # moving: separate tile so next block's GF can overlap this tail
        g3mov = blk.tile([P, 64 * Nk * Ng], F16, name="g3mov")
        nc.sync.dma_start(out=g3mov, in_=g3[h])
        egsb = blk.tile([P, 1024], F32, name="egsb")
        for sgrp in range(32):
            pt = psum.tile([P, 32], F32, name="peg")
            for sq in range(4):
                s = sgrp * 4 + sq
                for kgrp in range(4):
                    nc.tensor.matmul(
                        out=pt[32 * sq:32 * sq + 8, :],
                        lhsT=av(sta, (s * 4 + kgrp) * 8, [[1, 8]]),
                        rhs=av(g3mov, (s * 4 + kgrp) * 32, [[1, 32]]),
                        start=(kgrp == 0), stop=(kgrp == 3),
                        tile_position=(0, 32 * sq))
            nc.scalar.activation(egsb[:, sgrp * 32:(sgrp + 1) * 32], pt, AF.Copy)
        nc.sync.dma_start(out=estg[h], in_=egsb)
        # gather eg to sample-major egT [p, (c,n,r)]: per (oc, s4)
        e0 = estg[h]
        for oc in (0, 1):
            for s4 in range(4):
                nc.sync.dma_start(
                    out=bass.AP(tensor=egT.tensor,
                                offset=egT.offset + s4 * 256 + oc * 128,
                                ap=[[1024, 32], [4, 32], [1, 4]]),
                    in_=bass.AP(tensor=e0.tensor,
                                offset=e0.offset + (32 * s4 + oc) * 1024,
                                ap=[[32, 32], [1, 32], [2048, 4]]))

        # ---------- epilogue ----------
        if dbg is not None:
            nc.sync.dma_start(out=dbg["egT"][hs:hs + P], in_=egT)
        egr = egT[:, 0:128]
        egi = egT[:, 128:256]
        v.tensor_scalar_mul(egr, egr, -1.0 / Nk)
        v.tensor_scalar_mul(egi, egi, -1.0 / Nk)
        v.tensor_mul(egr, egr, pm_t)
        v.tensor_mul(egi, egi, pm_t)

        # step MLP (fixed leaky-relu)
        iq = blk.tile([P, 64], F32, name="iq")
        v.tensor_reduce(av(iq, 0, [[1, 32], [0, 1]]),
                        av(f32_t, 0, [[4, 32], [1, 4]]), axis=AX, op=ADD)
        v.tensor_reduce(av(iq, 32, [[1, 32], [0, 1]]),
                        av(f32_t, 128, [[4, 32], [1, 4]]), axis=AX, op=ADD)
        v.tensor_mul(iq, iq, bnsc)
        v.tensor_add(iq, iq, bnsh)
        v.tensor_mul(iq, iq, dw_t)
        z = blk.tile([P, 1], F32, name="z")
        v.tensor_reduce(z, iq.unsqueeze(1), axis=AX, op=ADD)
        v.tensor_add(z, z, db_t)
        smax = blk.tile([P, 1], F32, name="smax")
        step = blk.tile([P, 1], F32, name="step")
        v.tensor_scalar_max(smax, z, 0.0)
        v.tensor_scalar_min(step, z, 0.0)
        v.scalar_tensor_tensor(out=step, in0=step, scalar=ALPHA, in1=smax,
                               op0=MULT, op1=ADD)

        # proj = Re(eg * conj(Frf)); rg = eg - proj*Frf
        proj = blk.tile([P, 128], F32, name="proj")
        t0 = sc1[:, 0:128]
        t1 = sc2[:, 0:128]
        v.tensor_mul(proj, egr, fre)
        v.tensor_mul(t0, egi, fim)
        v.tensor_add(proj, proj, t0)
        rgr = blk.tile([P, 128], F32, name="rgr")
        rgi = blk.tile([P, 128], F32, name="rgi")
        v.tensor_mul(t0, proj, fre)
        v.tensor_sub(rgr, egr, t0)
        v.tensor_mul(t0, proj, fim)
        v.tensor_sub(rgi, egi, t0)

        # nrm; sc = -step/nrm
        n2 = blk.tile([P, 1], F32, name="n2")
        v.tensor_mul(t0, rgr, rgr)
        v.tensor_mul(t1, rgi, rgi)
        v.tensor_add(t0, t0, t1)
        v.tensor_reduce(n2, t0.unsqueeze(1), axis=AX, op=ADD)
        nc.scalar.activation(n2, n2, AF.Sqrt, bias=zero1)
        v.reciprocal(n2, n2)
        v.tensor_mul(n2, n2, step)
        v.tensor_scalar_mul(n2, n2, -1.0)
        fnr = blk.tile([P, 128], F32, name="fnr")
        fni = blk.tile([P, 128], F32, name="fni")
        v.scalar_tensor_tensor(out=fnr, in0=rgr, scalar=n2, in1=fre,
                               op0=MULT, op1=ADD)
        v.scalar_tensor_tensor(out=fni, in0=rgi, scalar=n2, in1=fim,
                               op0=MULT, op1=ADD)

        # scale = relu(|fnew|-1)+1 ; out = fnew/scale (interleaved)
        m2 = blk.tile([P, 128], F32, name="m2")
        v.tensor_mul(m2, fnr, fnr)
        v.tensor_mul(t0, fni, fni)
        v.tensor_add(m2, m2, t0)
        nc.scalar.activation(m2, m2, AF.Sqrt, bias=zero1)
        nc.scalar.activation(m2, m2, AF.Relu, bias=neg1)
        v.tensor_scalar_add(m2, m2, 1.0)
        v.reciprocal(m2, m2)
        ob = blk.tile([P, 256], F32, name="ob")
        v.tensor_mul(av(ob, 0, [[2, 128]]), fnr, m2)
        v.tensor_mul(av(ob, 1, [[2, 128]]), fni, m2)
        nc.sync.dma_start(out=out_v[hs:hs + P], in_=ob)


def inv4_batched(v, scr, S32, Y32):
    """Y32 = inv(S32) for Nk batched 4x4 complex mats, layout [k,c,i,j] f32.
    2x2 block Schur: S=[[A,B],[C,D]] -> iA, E=D-C iA B, iE, assemble."""
    KS = 32  # per-k stride

    # mini complex 2x2 tiles: layout [k, c(4), i(2), j(1)] width Nk*8
    mt = {nm: scr.tile([P, Nk * 8], F32, name="m_" + nm)
          for nm in ("iA", "iE", "P2", "Q2", "T2", "E2")}
    sc = scr.tile([P, Nk * 16], F32, name="m_sc")

    def sl(i):  # scratch slot [p, Nk]
        return av(sc, i * Nk, [[1, Nk]])

    def ent(t, base, c, i, j, mini=False):
        if mini:
            return av(t, c * 4 + i * 2 + j, [[8, Nk]])
        return av(t, base + c * 16 + i * 4 + j, [[KS, Nk]])

    def blkv(t, base, c, mini=False):
        if mini:
            return av(t, c * 4, [[8, Nk], [2, 2], [1, 2]])
        return av(t, base + c * 16, [[KS, Nk], [4, 2], [1, 2]])

    def c2inv(dst, src, sbase, smini):
        """dst (mini) = inv of 2x2 complex block of src at sbase."""
        e = lambda c, i, j: ent(src, sbase, c, i, j, smini)
        dre, dim, q0, q1 = sl(0), sl(1), sl(2), sl(3)
        # det = s00*s11 - s01*s10
        v.tensor_mul(dre, e(0, 0, 0), e(0, 1, 1))
        v.tensor_mul(q0, e(1, 0, 0), e(1, 1, 1))
        v.tensor_sub(dre, dre, q0)
        v.tensor_mul(q0, e(0, 0, 1), e(0, 1, 0))
        v.tensor_sub(dre, dre, q0)
        v.tensor_mul(q0, e(1, 0, 1), e(1, 1, 0))
        v.tensor_add(dre, dre, q0)
        v.tensor_mul(dim, e(0, 0, 0), e(1, 1, 1))
        v.tensor_mul(q0, e(1, 0, 0), e(0, 1, 1))
        v.tensor_add(dim, dim, q0)
        v.tensor_mul(q0, e(0, 0, 1), e(1, 1, 0))
        v.tensor_sub(dim, dim, q0)
        v.tensor_mul(q0, e(1, 0, 1), e(0, 1, 0))
        v.tensor_sub(dim, dim, q0)
        # inv det (conj form)
        v.tensor_mul(q0, dre, dre)
        v.tensor_mul(q1, dim, dim)
        v.tensor_add(q0, q0, q1)
        v.reciprocal(q0, q0)
        v.tensor_mul(dre, dre, q0)
        v.tensor_mul(dim, dim, q0)
        v.tensor_scalar_mul(dim, dim, -1.0)
        # adj entries into sc slots 4..11: [a11,-a01,-a10,a00] per comp
        for c in (0, 1):
            base_sl = 4 + c * 4
            v.tensor_copy(sl(base_sl + 0), e(c, 1, 1))
            v.tensor_scalar_mul(sl(base_sl + 1), e(c, 0, 1), -1.0)
            v.tensor_scalar_mul(sl(base_sl + 2), e(c, 1, 0), -1.0)
            v.tensor_copy(sl(base_sl + 3), e(c, 0, 0))
        adjr = av(sc, 4 * Nk, [[1, Nk], [2 * Nk, 2], [Nk, 2]])
        adji = av(sc, 8 * Nk, [[1, Nk], [2 * Nk, 2], [Nk, 2]])
        q2 = av(sc, 12 * Nk, [[1, Nk], [2 * Nk, 2], [Nk, 2]])
        dreb = av(sc, 0, [[1, Nk], [0, 2], [0, 2]])
        dimb = av(sc, Nk, [[1, Nk], [0, 2], [0, 2]])
        dr = blkv(dst, 0, 0, True)
        di = blkv(dst, 0, 1, True)
        # re = adjr*dre - adji*dim ; im = adjr*dim + adji*dre
        v.tensor_mul(q2, adjr, dreb)
        v.tensor_mul(dr, adji, dimb)
        v.tensor_sub(dr, q2, dr)
        v.tensor_mul(q2, adjr, dimb)
        v.tensor_mul(di, adji, dreb)
        v.tensor_add(di, q2, di)

    def c2mul(dst, x, xbase, xmini, y, ybase, ymini,
              sub_from=None, sf_base=0, sf_mini=False, negate=False):
        """dst (mini) = x@y | sub_from - x@y | -(x@y), 2x2 complex."""
        t0f = av(sc, 0, [[1, Nk], [Nk, 8]])
        t1f = av(sc, 8 * Nk, [[1, Nk], [Nk, 8]])

        def bxr(t, base, c, i, mini):
            # row i of x block: [k, j(bcast), m]
            if mini:
                return av(t, c * 4 + i * 2, [[8, Nk], [0, 2], [1, 2]])
            return av(t, base + c * 16 + i * 4, [[KS, Nk], [0, 2], [1, 2]])

        def by(t, base, c, mini):
            # y block as [k, j, m] with entry (m,j)
            if mini:
                return av(t, c * 4, [[8, Nk], [1, 2], [2, 2]])
            return av(t, base + c * 16, [[KS, Nk], [1, 2], [4, 2]])

        for comp in (0, 1):
            for i in (0, 1):
                t0r = av(sc, i * 4 * Nk, [[1, Nk], [2 * Nk, 2], [Nk, 2]])
                t1r = av(sc, (8 + i * 4) * Nk, [[1, Nk], [2 * Nk, 2], [Nk, 2]])
                v.tensor_mul(t0r, bxr(x, xbase, 0, i, xmini), by(y, ybase, comp, ymini))
                v.tensor_mul(t1r, bxr(x, xbase, 1, i, xmini), by(y, ybase, 1 - comp, ymini))
            if comp == 0:
                v.tensor_sub(t0f, t0f, t1f)
            else:
                v.tensor_add(t0f, t0f, t1f)
            d = blkv(dst, 0, comp, True)
            red = av(sc, 8 * Nk, [[1, Nk], [Nk, 4], [0, 1]])
            v.tensor_reduce(red, av(sc, 0, [[1, Nk], [2 * Nk, 4], [Nk, 2]]),
                            axis=AX, op=ADD)
            redv = av(sc, 8 * Nk, [[1, Nk], [2 * Nk, 2], [Nk, 2]])
            if sub_from is not None:
                v.tensor_sub(d, blkv(sub_from, sf_base, comp, sf_mini), redv)
            elif negate:
                v.tensor_scalar_mul(d, redv, -1.0)
            else:
                v.tensor_copy(d, redv)

    iA, iE, P2, Q2, T2, E2 = (mt[n] for n in ("iA", "iE", "P2", "Q2", "T2", "E2"))
    # S blocks: A off 0, B off 2, C off 8, D off 10
    c2inv(iA, S32, 0, False)
    c2mul(P2, iA, 0, True, S32, 2, False)                 # P2 = iA@B
    c2mul(Q2, S32, 8, False, iA, 0, True)                 # Q2 = C@iA
    c2mul(E2, S32, 8, False, P2, 0, True,
          sub_from=S32, sf_base=10, sf_mini=False)        # E2 = D - C@P2
    c2inv(iE, E2, 0, True)
    c2mul(T2, P2, 0, True, iE, 0, True, negate=True)      # T2 = -P2@iE = Y12
    # write Y12 -> Y32[0:2,2:4]
    for c in (0, 1):
        v.tensor_copy(blkv(Y32, 2, c), blkv(T2, 0, c, True))
    c2mul(E2, iE, 0, True, Q2, 0, True, negate=True)      # E2 = -iE@Q2 = Y21
    for c in (0, 1):
        v.tensor_copy(blkv(Y32, 8, c), blkv(E2, 0, c, True))
    # Y11 = iA - Y12@Q2  (T2 holds Y12; write to E2 to avoid dst/src alias)
    c2mul(E2, T2, 0, True, Q2, 0, True,
          sub_from=iA, sf_base=0, sf_mini=True)
    for c in (0, 1):
        v.tensor_copy(blkv(Y32, 0, c), blkv(E2, 0, c, True))
    for c in (0, 1):
        v.tensor_copy(blkv(Y32, 10, c), blkv(iE, 0, c, True))


# ================= host side =================

_NC_CACHE = {}


def _prep(inputs):
    f16, f32 = np.float16, np.float32
    Gr = np.asarray(inputs["G_re"], f32)
    Gi = np.asarray(inputs["G_im"], f32)
    Lr = np.asarray(inputs["Lam_re"], f32)
    Li = np.asarray(inputs["Lam_im"], f32)
    fr = np.asarray(inputs["Frf_re"], f32)
    fi = np.asarray(inputs["Frf_im"], f32)


    lamx = np.empty((Nk, B, Ng, 3, Ng), f16)    # Lam 3-slot: [g,s,g']
    lamx[..., 0, :] = Lr + Li
    lamx[..., 1, :] = Lr
    lamx[..., 2, :] = Li
    frT = fr.transpose(0, 2, 1)                  # [B,r,n]
    fiT = fi.transpose(0, 2, 1)
    fb4 = np.empty((B, 2, Nrf, 2, Nt), f16)     # B4(Frf): [oc,r,c,n]
    fb4[:, 0, :, 0, :] = frT
    fb4[:, 0, :, 1, :] = -fiT
    fb4[:, 1, :, 0, :] = fiT
    fb4[:, 1, :, 1, :] = frT
    f32x = np.empty((B, 2, Nt, Nrf), f32)
    f32x[:, 0] = fr
    f32x[:, 1] = fi
    u32 = np.ascontiguousarray(
        (1.0 / np.asarray(inputs["Beta_re"], f32).reshape(Nk, B).T))
    aux = np.concatenate([
        np.asarray(inputs["bn_gamma"], f32).ravel(),
        np.asarray(inputs["bn_beta"], f32).ravel(),
        np.asarray(inputs["bn_mean"], f32).ravel(),
        np.asarray(inputs["bn_var"], f32).ravel(),
        np.asarray(inputs["dense_w"], f32).ravel(),
        np.asarray(inputs["dense_b"], f32).ravel(),
        np.asarray(inputs["P_mask"], f32).ravel()])

    in_maps = []
    for c in range(NCORES):
        s = slice(c * BL, (c + 1) * BL)
        # gpe: moving G for PE-GF: [h, row=64s+32cc+n, pair*256 + k*16+g]
        Xc = np.stack([Gr[:, s], Gi[:, s]])          # [cc,k,BL,n,g]
        Xc = Xc.reshape(2, Nk, H, 64, 2, Nt, Ng)     # [cc,k,h,q,ss,n,g]
        gpe = np.ascontiguousarray(
            Xc.transpose(2, 4, 0, 5, 3, 1, 6).reshape(H, P, 64 * Nk * Ng)
        ).astype(f16)
        # fpe: block-diag stationary: rows as gpe, cols pair*16 + 8s+2r+oc
        # oc=0 (re): (cc0: fr, cc1: fi) ; oc=1 (-im, conj): (cc0: -fi, cc1: fr)
        frc = fr[s].reshape(H, 64, 2, Nt, Nrf)       # [h,q,ss,n,r]
        fic = fi[s].reshape(H, 64, 2, Nt, Nrf)
        fpe_a = np.zeros((H, 2, 2, Nt, 64, 16), np.float32)  # [h,ss,cc,n,q,col]
        for ss in (0, 1):
            fq = frc[:, :, ss].transpose(0, 2, 1, 3)  # [h,n,q,r]
            gq = fic[:, :, ss].transpose(0, 2, 1, 3)
            cb = np.empty((2, H, Nt, 64, Nrf, 2), np.float32)  # [cc,h,n,q,r,oc]
            cb[0, ..., 0] = fq
            cb[0, ..., 1] = -gq
            cb[1, ..., 0] = gq
            cb[1, ..., 1] = fq
            for cc in (0, 1):
                fpe_a[:, ss, cc, :, :, 8 * ss:8 * ss + 8] = \
                    cb[cc].reshape(H, Nt, 64, 8)
        fpe = np.ascontiguousarray(fpe_a.reshape(H, P, 64 * 16)).astype(f16)
        # g3: eg moving operand: rows 32*kq+2g+cc, cols (sample*4+kgrp)*32+n
        Yc = np.stack([Gr[:, s], Gi[:, s]])          # [cc,k,BL,n,g]
        Yc = Yc.reshape(2, 4, 4, H, P, Nt, Ng)       # [cc,kgrp,kq,h,smp,n,g]
        g3 = np.ascontiguousarray(
            Yc.transpose(3, 2, 6, 0, 4, 1, 5).reshape(H, P, 32 * Nk * Nt)
        ).astype(f16)
        in_maps.append({
            "gpe": gpe,
            "fpe": fpe,
            "g3": g3,
            "lam": np.ascontiguousarray(lamx[:, s]).reshape(Nk, BL, WLAM),
            "fb4": np.ascontiguousarray(fb4[s]).reshape(BL, 512),
            "f32t": np.ascontiguousarray(f32x[s]).reshape(BL, 256),
            "u32": np.ascontiguousarray(u32[s]),
            "aux": aux,
        })
    return in_maps


def kernel(**inputs):
    in_maps = _prep(inputs)
    if "nc" not in _NC_CACHE:
        _NC_CACHE["nc"] = build_nc()
    nc = _NC_CACHE["nc"]
    from concourse.bass_utils import run_bass_kernel_spmd
    res = run_bass_kernel_spmd(nc, in_maps, core_ids=list(range(NCORES)))
    outs = [res.results[c]["out"] for c in range(NCORES)]
    full = np.concatenate(outs, axis=0).reshape(B, Nt, Nrf, 2)
    return np.ascontiguousarray(full).view(np.complex64).reshape(B, Nt, Nrf)


def kernel_profiled(**inputs):
    """Returns modeled HW exec time (ns) from the cost-model timeline sim."""
    if "nc" not in _NC_CACHE:
        _NC_CACHE["nc"] = build_nc()
    from concourse.timeline_sim import TimelineSim
    ts = TimelineSim(_NC_CACHE["nc"], no_exec=True)
    return int(ts.simulate())


# revision 57
# speedup vs baseline: 1.0622x; 1.0051x over previous
"""Trainium2 Bass kernel for nn_DUP_block_90391881712206.

Math per (k,b) via Woodbury (no 16x16 inversions):
    GF = conj(Gh)^T @ Frf            [Ng,Nrf]
    T' = Lam @ GF
    S  = I + u * GF^H @ T'  (u = 1/beta)
    Y  = inv(S)  (4x4 complex, 2x2 block Schur, f32)
    V  = u * T' @ Y
    C  = T'^H @ V ; D = u * Y @ C
    E  = T' @ D ; W = Lam @ V ; mgf = W - E
    eg_k = Gh @ mgf
eg = -mean_k(eg_k) * P_mask, then Riemannian step + unit-disk clamp.

Implementation: data-parallel over B across 8 cores (256 b/core, 2 blocks
of 128 partitions). Batched small complex matmuls on DVE in fp16: one
broadcast-AP multiply builds the product tensor [outc,d1,d2,c,j] (packed
last dim -> 2x DVE mode), then log2 halving-fold adds reduce segments
(packed -> 2x). Host pre-lays inputs (conj baked into G's imag for GF).
"""

import numpy as np
import sys
from contextlib import ExitStack

sys.path.insert(0, "/opt/trn_rl_repo")

import concourse.bass as bass
import concourse.bacc as bacc_mod
import concourse.tile as tile
from concourse import mybir

Nk, B, Nt, Nrf, Ng = 16, 2048, 32, 4, 16
NCORES = 8
BL = B // NCORES
P = 128
H = BL // P
KC = 2                     # k per chunk
NCH = Nk // KC
BN_EPS = 1e-3
ALPHA = 0.1

F16 = mybir.dt.float16
F32 = mybir.dt.float32
AX = mybir.AxisListType.X
ADD = mybir.AluOpType.add
SUB = mybir.AluOpType.subtract
MULT = mybir.AluOpType.mult
AF = mybir.ActivationFunctionType

# per-k widths (fp16 elems)
WG1 = Ng * 2 * Nt          # 1024  g1 [g,c,n]
WG2 = Nt * 2 * Ng          # 1024  g2 [n,c,g]
WLAM = Ng * 3 * Ng         # 768   lam 3-slot [g,s,g'] (Karatsuba)


def av(t, off, dims):
    """AP view of tile t at free-offset `off` with free dims [[stride,n],..]."""
    return bass.AP(tensor=t.tensor, offset=t.offset + off,
                   ap=[list(t.ap[0])] + [list(d) for d in dims])


def build_nc(debug=False):
    nc = bacc_mod.Bacc()
    gpe = nc.dram_tensor("gpe", [H, P, 64 * Nk * Ng], F16, kind="ExternalInput")
    fpe = nc.dram_tensor("fpe", [H, P, 64 * 16], F16, kind="ExternalInput")
    stg = nc.dram_tensor("stg", [H, 16, P, 256], F16, kind="Internal")
    g3 = nc.dram_tensor("g3", [H, P, 32 * Nk * Nt], F16, kind="ExternalInput")
    mstg = nc.dram_tensor("mstg", [H, Nk, P, 256], F16, kind="Internal")
    estg = nc.dram_tensor("estg", [H, P, 1024], F32, kind="Internal")
    lam = nc.dram_tensor("lam", [Nk, BL, WLAM], F16, kind="ExternalInput")
    fb4 = nc.dram_tensor("fb4", [BL, 2 * Nrf * 2 * Nt], F16, kind="ExternalInput")
    f32t = nc.dram_tensor("f32t", [BL, 2 * Nt * Nrf], F32, kind="ExternalInput")
    u32 = nc.dram_tensor("u32", [BL, Nk], F32, kind="ExternalInput")
    aux = nc.dram_tensor("aux", [5 * 2 * Nt + 1 + Nt * Nrf], F32, kind="ExternalInput")
    out = nc.dram_tensor("out", [BL, Nt * Nrf * 2], F32, kind="ExternalOutput")
    dbg = None
    if debug:
        dbg = {nm: nc.dram_tensor("dbg_" + nm, [BL, w], dt, kind="ExternalOutput")
               for nm, w, dt in (("a3s", Nk * 128, F16), ("a3ve", Nk * 128, F16),
                                 ("S32", Nk * 32, F32), ("Y32", Nk * 32, F32),
                                 ("vw", Nk * 256, F16), ("keg", Nk * 256, F16),
                                 ("egT", 256, F32))}

    with ExitStack() as ctx:
        tc = ctx.enter_context(tile.TileContext(nc))
        kern(ctx, tc, gpe, fpe, stg, g3, mstg, estg, lam, fb4, f32t, u32, aux,
             out, dbg)
    if not nc.is_finalized():
        nc.finalize()
    return nc


def kern(ctx, tc, gpe, fpe, stg, g3, mstg, estg, lam, fb4, f32t, u32, aux,
         out, dbg=None):
    nc = tc.nc
    v = nc.vector

    singles = ctx.enter_context(tc.tile_pool(name="singles", bufs=1))
    loads = ctx.enter_context(tc.tile_pool(name="loads", bufs=3))
    blk = ctx.enter_context(tc.tile_pool(name="blk", bufs=1))
    scr = ctx.enter_context(tc.tile_pool(name="scr", bufs=1))
    psum = ctx.enter_context(tc.tile_pool(name="psum", bufs=4, space="PSUM"))

    # ---- broadcast-load aux params ----
    aux_t = singles.tile([P, 449], F32, name="aux_bc")
    aux_ap = aux[:]
    src = bass.AP(tensor=aux_ap.tensor, offset=aux_ap.offset,
                  ap=[[0, P]] + list(aux_ap.ap))
    nc.gpsimd.dma_start(out=aux_t, in_=src)
    gam_t, bb_t = aux_t[:, 0:64], aux_t[:, 64:128]
    bm_t, bv_t = aux_t[:, 128:192], aux_t[:, 192:256]
    dw_t, db_t = aux_t[:, 256:320], aux_t[:, 320:321]
    pm_t = aux_t[:, 321:449]

    zero1 = singles.tile([P, 1], F32, name="zero1")
    neg1 = singles.tile([P, 1], F32, name="neg1")
    v.memset(zero1, 0.0)
    v.memset(neg1, -1.0)
    eye16 = singles.tile([P, 16], F32, name="eye16")
    v.memset(eye16, 0.0)
    v.memset(av(eye16, 0, [[5, 4]]), 1.0)

    # bn scale/shift
    bnsc = singles.tile([P, 64], F32, name="bnsc")
    bnsh = singles.tile([P, 64], F32, name="bnsh")
    v.tensor_scalar_add(bnsc, bv_t, BN_EPS)
    v.reciprocal(bnsc, bnsc)
    nc.scalar.activation(bnsc, bnsc, AF.Sqrt, bias=zero1)
    v.tensor_mul(bnsc, bnsc, gam_t)
    v.tensor_mul(bnsh, bm_t, bnsc)
    v.tensor_sub(bnsh, bb_t, bnsh)

    # ---- scratch tiles ----
    # product tensors + fold ping-pong (sized for biggest cmat: 4w=8192/k)
    Pt = scr.tile([P, KC * 4096], F16, name="Pt")
    Pf1 = scr.tile([P, KC * 2048], F16, name="Pf1")
    Pf2 = scr.tile([P, KC * 1024], F16, name="Pf2")
    sc0 = scr.tile([P, KC * 256], F32, name="sc0")   # f32 scratch (D etc)
    sc1 = scr.tile([P, 256], F32, name="sc1")
    sc2 = scr.tile([P, 256], F32, name="sc2")

    def fold_reduce(src_t, src_off, nseg, L, eng1=None):
        """Sum contiguous segments of length L (pow2) via halving adds.
        Returns (tile, offset) of compact [nseg] result. eng1: engine for
        the first (widest) fold level (e.g. nc.gpsimd to offload)."""
        cur_t, cur_off, cl = src_t, src_off, L
        ping = [Pf1, Pf2]
        pi = 0
        first = True
        while cl > 1:
            half = cl // 2
            dst = ping[pi]
            pi ^= 1
            eng = eng1 if (first and eng1 is not None) else v
            eng.tensor_add(av(dst, 0, [[half, nseg], [1, half]]),
                           av(cur_t, cur_off, [[cl, nseg], [1, half]]),
                           av(cur_t, cur_off + half, [[cl, nseg], [1, half]]))
            cur_t, cur_off, cl = dst, 0, half
            first = False
        return cur_t, cur_off

    def cmat(a_t, a_off, a_d1s, a_cs, b4_t, b4_off, b4_ocs, b4_d2s, b4_cs,
             d1, d2, j, p_off=0):
        """Product P[outc,d1,d2,c,j] = A[d1,c,j] * B4[outc,d2,c,j], then
        fold-reduce (c,j) -> K [outc,d1,d2] compact fp16. One k at a time
        for the mul (4-dim APs); returns nothing (call fold separately)."""
        L = 2 * j
        w2 = d1 * d2 * L
        a_v = av(a_t, a_off, [[a_d1s, d1], [0, d2], [1, L]])
        for oc in range(2):
            ov = av(Pt, p_off + oc * w2, [[d2 * L, d1], [L, d2], [1, L]])
            b_v = av(b4_t, b4_off + oc * b4_ocs, [[0, d1], [b4_d2s, d2], [1, L]])
            v.tensor_mul(ov, a_v, b_v)

    def cmat3(a_t, a_off, a_d1s, a_ss, b3_t, b3_off, b3_ss, b3_d2s,
              d1, d2, j, p_off=0):
        """Gauss 3-mult: P3[s,d1,d2,j] = A3[d1,s,j] * B3[s,d2,j];
        fold j -> K [s,d1,d2]; re=K1-K3, im=K1+K2."""
        w1 = d1 * d2 * j
        for sl in range(3):
            ov = av(Pt, p_off + sl * w1, [[d2 * j, d1], [j, d2], [1, j]])
            a_v = av(a_t, a_off + sl * a_ss, [[a_d1s, d1], [0, d2], [1, j]])
            b_v = av(b3_t, b3_off + sl * b3_ss, [[0, d1], [b3_d2s, d2], [1, j]])
            v.tensor_mul(ov, a_v, b_v)

    # ============ per-block main ============
    out_v = out
    for h in range(H):
        hs = h * P

        fb4_t = blk.tile([P, 512], F16, name="fb4_t")
        f32_t = blk.tile([P, 256], F32, name="f32_t")
        u_t = blk.tile([P, Nk], F32, name="u_t")
        nc.sync.dma_start(out=fb4_t, in_=fb4[hs:hs + P])
        nc.sync.dma_start(out=f32_t, in_=f32t[hs:hs + P])
        nc.sync.dma_start(out=u_t, in_=u32[hs:hs + P])
        fre = f32_t[:, 0:128]
        fim = f32_t[:, 128:256]

        lam_t = blk.tile([P, Nk * WLAM], F16, name="lam_t")
        nc.sync.dma_start(out=lam_t.rearrange("p (k w) -> p k w", k=Nk),
                          in_=lam[:, hs:hs + P].rearrange("k b w -> b k w"))

        # all-k mid tensors (fp16 per-k layouts)
        a3ve = blk.tile([P, Nk * 128], F16, name="a3ve")  # T' [g,c,r]
        a3c = blk.tile([P, Nk * 128], F16, name="a3c")    # conj(T') [r,c,g']
        b4s = blk.tile([P, Nk * 256], F16, name="b4s")    # B4(T') [oc,r2,c,g']
        a3s = blk.tile([P, Nk * 128], F16, name="a3s")    # conj(GF) [r,c,g']
        b3t = blk.tile([P, Nk * 192], F16, name="b3t")    # B3(GF) [s,r,g']
        b3w = blk.tile([P, KC * 192], F16, name="b3w")    # B3(V) [s,r2,g']
        S32 = blk.tile([P, Nk * 32], F32, name="S32")     # [k,c,i,j]
        Y32 = blk.tile([P, Nk * 32], F32, name="Y32")
        Yu32 = blk.tile([P, Nk * 32], F32, name="Yu32")
        b4y = blk.tile([P, Nk * 64], F16, name="b4y")     # B4(Yu) [oc,r2,c,r]
        b4cw = blk.tile([P, KC * 256], F16, name="b4cw")  # B4(V) [oc,r2,c,g']
        C32 = blk.tile([P, KC * 32], F32, name="C32")
        D32 = blk.tile([P, KC * 32], F32, name="D32")
        b4e = blk.tile([P, KC * 64], F16, name="b4e")     # B4(D) [oc,r2,c,r]
        ke = blk.tile([P, KC * 128], F16, name="ke")      # saved K of E
        egT = blk.tile([P, 256], F32, name="egT")
        mgf_t = blk.tile([P, Nk * 256], F16, name="mgf_t")

        # ---------- PE: GF for all k ----------
        # stationary fpe [128=(s,cc,n), 16=(s,r,oc)] per pair (block-diag Frf);
        # moving gmov [128, 256=(k,g)] per pair; out psum [16, 256] per pair.
        gmov = blk.tile([P, 64 * Nk * Ng], F16, name="gmov")
        fpe_t = blk.tile([P, 64 * 16], F16, name="fpe_t")
        nc.sync.dma_start(out=gmov, in_=gpe[h])
        nc.sync.dma_start(out=fpe_t, in_=fpe[h])
        for grp in range(16):
            pt = psum.tile([P, 256], F32, name="pgf")
            for qq in range(4):
                pair = grp * 4 + qq
                nc.tensor.matmul(out=pt[32 * qq:32 * qq + 16, :],
                                 lhsT=fpe_t[:, pair * 16:(pair + 1) * 16],
                                 rhs=av(gmov, pair * 256, [[1, 256]]),
                                 start=True, stop=True,
                                 tile_position=(0, 32 * qq))
            sgf = loads.tile([P, 256], F16, name="sgf")
            nc.scalar.activation(sgf, pt, AF.Copy)
            nc.sync.dma_start(out=stg[h, grp], in_=sgf)
        # stg row (per grp): 32*qq + 8s + 2r + oc ; col = k*16+g.
        # gather to a3s [p=sample, (r,c,k,g')]: 1 DMA per s
        st0 = stg[h, 0]
        pstride = Nk * 128
        for s in (0, 1):
            for r in range(4):
                for c in range(2):
                    nc.sync.dma_start(
                        out=bass.AP(tensor=a3s.tensor,
                                    offset=a3s.offset + s * pstride + r * 32 + c * 16,
                                    ap=[[2 * pstride, 64], [128, 16], [1, 16]]),
                        in_=bass.AP(tensor=st0.tensor,
                                    offset=st0.offset + s * 2048 + r * 512 + c * 256,
                                    ap=[[8192, 64], [16, 16], [1, 16]]))

        # b3t = B3(GF) slots (Gre, Gim-Gre, Gre+Gim); a3s = (Gre, -Gim)
        v.tensor_copy(av(b3t, 0, [[192, Nk], [16, 4], [1, 16]]),
                      av(a3s, 0, [[128, Nk], [32, 4], [1, 16]]))
        v.scalar_tensor_tensor(
            out=av(b3t, 64, [[192, Nk], [16, 4], [1, 16]]),
            in0=av(a3s, 16, [[128, Nk], [32, 4], [1, 16]]), scalar=-1.0,
            in1=av(a3s, 0, [[128, Nk], [32, 4], [1, 16]]),
            op0=MULT, op1=SUB)
        v.tensor_sub(av(b3t, 128, [[192, Nk], [16, 4], [1, 16]]),
                     av(a3s, 0, [[128, Nk], [32, 4], [1, 16]]),
                     av(a3s, 16, [[128, Nk], [32, 4], [1, 16]]))

        # ---------- pass 1: T', S per k-chunk ----------
        for ch in range(NCH):
            k0 = ch * KC

            # T' = Lam @ GF (Gauss): A3=lam [g,s,g'], B3=b3t; d1=g,d2=r,j=g'
            for kk in range(KC):
                cmat3(lam_t, (k0 + kk) * WLAM, 48, 16, b3t, (k0 + kk) * 192, 64, 16,
                      Ng, Nrf, Ng, p_off=kk * 3072)
            kt, ko = fold_reduce(Pt, 0, KC * 3 * Ng * Nrf, 16)
            # K [kk, s, g, r] strides 192,64,4,1 ; re=K1-K3, im=K1+K2
            kg0 = k0
            K1 = av(kt, ko, [[192, KC], [4, 16], [1, 4]])
            K2 = av(kt, ko + 64, [[192, KC], [4, 16], [1, 4]])
            K3 = av(kt, ko + 128, [[192, KC], [4, 16], [1, 4]])
            K1t = av(kt, ko, [[192, KC], [1, 4], [4, 16]])
            K2t = av(kt, ko + 64, [[192, KC], [1, 4], [4, 16]])
            K3t = av(kt, ko + 128, [[192, KC], [1, 4], [4, 16]])
            # a3ve = T' [g,c,r]
            v.tensor_sub(av(a3ve, kg0 * 128, [[128, KC], [8, 16], [1, 4]]), K1, K3)
            v.tensor_add(av(a3ve, kg0 * 128 + 4, [[128, KC], [8, 16], [1, 4]]), K1, K2)
            # a3c = conj(T') [r,c,g']
            v.tensor_sub(av(a3c, kg0 * 128, [[128, KC], [32, 4], [1, 16]]), K1t, K3t)
            a3c_im = av(a3c, kg0 * 128 + 16, [[128, KC], [32, 4], [1, 16]])
            v.tensor_add(a3c_im, K1t, K2t)
            v.tensor_scalar_mul(a3c_im, a3c_im, -1.0)
            # b4s = B4(T') [oc,r2,c,g']
            v.tensor_sub(av(b4s, kg0 * 256, [[256, KC], [32, 4], [1, 16]]), K1t, K3t)
            v.tensor_copy(av(b4s, kg0 * 256 + 16, [[256, KC], [32, 4], [1, 16]]),
                          a3c_im)
            v.tensor_add(av(b4s, kg0 * 256 + 128, [[256, KC], [32, 4], [1, 16]]), K1t, K2t)
            v.tensor_sub(av(b4s, kg0 * 256 + 144, [[256, KC], [32, 4], [1, 16]]), K1t, K3t)

            # S' = GF^H T': A=a3s [r1,c,g'], B4=b4s; d1=r1,d2=r2,j=g'
            for kk in range(KC):
                cmat(a3s, (k0 + kk) * 128, 32, 16, b4s, (k0 + kk) * 256, 128, 32, 16,
                     Nrf, Nrf, Ng, p_off=kk * 1024)
            kt, ko = fold_reduce(Pt, 0, KC * 2 * Nrf * Nrf, 32)
            # K [kk, outc, r1, r2] strides 32,16,4,1 -> S32 [k,c,i,j]
            v.tensor_copy(av(S32, k0 * 32, [[32, KC], [16, 2], [1, 16]]),
                          av(kt, ko, [[32, KC], [16, 2], [1, 16]]))
            # S = I + u*S'
            v.tensor_mul(av(S32, k0 * 32, [[32, KC], [1, 32]]),
                         av(S32, k0 * 32, [[32, KC], [1, 32]]),
                         av(u_t, k0, [[1, KC], [0, 32]]))
            v.tensor_add(av(S32, k0 * 32, [[32, KC], [1, 16]]),
                         av(S32, k0 * 32, [[32, KC], [1, 16]]),
                         av(eye16, 0, [[0, KC], [1, 16]]))

        if dbg is not None:
            nc.sync.dma_start(out=dbg["a3s"][hs:hs + P], in_=a3s)
            nc.sync.dma_start(out=dbg["a3ve"][hs:hs + P], in_=a3ve)
            nc.sync.dma_start(out=dbg["S32"][hs:hs + P], in_=S32)

        # ---------- pass 2: batched 4x4 inversion (2x2 Schur), f32 ----------
        inv4_batched(v, scr, S32, Y32)
        # Yu = u*Y
        v.tensor_mul(av(Yu32, 0, [[32, Nk], [1, 32]]),
                     av(Y32, 0, [[32, Nk], [1, 32]]),
                     av(u_t, 0, [[1, Nk], [0, 32]]))
        # b4y = B4(Yu) [oc,r2,c,r], B[r2,r]=Yu[r,r2] (transposed)
        for (qoff, soff, sgn) in ((0, 0, 1.0), (4, 16, -1.0),
                                  (32, 16, 1.0), (36, 0, 1.0)):
            if sgn > 0:
                v.tensor_copy(av(b4y, qoff, [[64, Nk], [8, 4], [1, 4]]),
                              av(Yu32, soff, [[32, Nk], [1, 4], [4, 4]]))
            else:
                v.tensor_scalar_mul(av(b4y, qoff, [[64, Nk], [8, 4], [1, 4]]),
                                    av(Yu32, soff, [[32, Nk], [1, 4], [4, 4]]), -1.0)

        if dbg is not None:
            nc.sync.dma_start(out=dbg["Y32"][hs:hs + P], in_=Y32)

        # ---------- pass 3: V, C, D, E, W, mgf, eg per k-chunk ----------
        for ch in range(NCH):
            k0 = ch * KC

            # V = T' @ Yu: A=a3ve [g,c,r], B4=b4y; d1=g,d2=r2,j=r
            for kk in range(KC):
                cmat(a3ve, (k0 + kk) * 128, 8, 4, b4y, (k0 + kk) * 64, 32, 8, 4,
                     Ng, Nrf, Nrf, p_off=kk * 1024)
            ktv, kov = fold_reduce(Pt, 0, KC * 2 * Ng * Nrf, 8)
            kt, ko = ktv, kov
            # K [kk,outc,g,r2] strides 128,64,4,1 -> b4cw [oc,r2,c,g'](V)
            v.tensor_copy(av(b4cw, 0, [[256, KC], [32, 4], [1, 16]]),
                          av(kt, ko, [[128, KC], [1, 4], [4, 16]]))
            v.tensor_scalar_mul(av(b4cw, 16, [[256, KC], [32, 4], [1, 16]]),
                                av(kt, ko + 64, [[128, KC], [1, 4], [4, 16]]), -1.0)
            v.tensor_copy(av(b4cw, 128, [[256, KC], [32, 4], [1, 16]]),
                          av(kt, ko + 64, [[128, KC], [1, 4], [4, 16]]))
            v.tensor_copy(av(b4cw, 144, [[256, KC], [32, 4], [1, 16]]),
                          av(kt, ko, [[128, KC], [1, 4], [4, 16]]))

            # W = Lam @ V (Gauss): B3(V) slots (Vre,Vim-Vre,Vre+Vim) [s,r2,g']
            v.tensor_copy(av(b3w, 0, [[192, KC], [16, 4], [1, 16]]),
                          av(ktv, kov, [[128, KC], [1, 4], [4, 16]]))
            v.tensor_sub(av(b3w, 64, [[192, KC], [16, 4], [1, 16]]),
                         av(ktv, kov + 64, [[128, KC], [1, 4], [4, 16]]),
                         av(ktv, kov, [[128, KC], [1, 4], [4, 16]]))
            v.tensor_add(av(b3w, 128, [[192, KC], [16, 4], [1, 16]]),
                         av(ktv, kov, [[128, KC], [1, 4], [4, 16]]),
                         av(ktv, kov + 64, [[128, KC], [1, 4], [4, 16]]))

            if dbg is not None:
                nc.sync.dma_start(out=dbg["vw"][hs:hs + P, k0 * 256:(k0 + KC) * 256],
                                  in_=b4cw)

            # C = T'^H @ V: A=a3c [r1,c,g'], B4=b4cw; d1=r1,d2=r2,j=g'
            for kk in range(KC):
                cmat(a3c, (k0 + kk) * 128, 32, 16, b4cw, kk * 256, 128, 32, 16,
                     Nrf, Nrf, Ng, p_off=kk * 1024)
            kt, ko = fold_reduce(Pt, 0, KC * 2 * Nrf * Nrf, 32)
            v.tensor_copy(av(C32, 0, [[32, KC], [16, 2], [1, 16]]),
                          av(kt, ko, [[32, KC], [16, 2], [1, 16]]))

            # D = u * Y @ C (f32). product dims per k: [r1, r2, m]
            for comp in (0, 1):
                for kk in range(KC):
                    yv = lambda c: av(Y32, (k0 + kk) * 32 + c * 16,
                                      [[4, 4], [0, 4], [1, 4]])
                    cv = lambda c: av(C32, kk * 32 + c * 16,
                                      [[0, 4], [1, 4], [4, 4]])
                    t0 = av(sc0, kk * 64, [[16, 4], [4, 4], [1, 4]])
                    t1 = av(sc0, KC * 64 + kk * 64, [[16, 4], [4, 4], [1, 4]])
                    v.tensor_mul(t0, yv(0), cv(comp))
                    v.tensor_mul(t1, yv(1), cv(1 - comp))
                t0f = av(sc0, 0, [[1, KC * 64]])
                t1f = av(sc0, KC * 64, [[1, KC * 64]])
                if comp == 0:
                    v.tensor_sub(t0f, t0f, t1f)
                else:
                    v.tensor_add(t0f, t0f, t1f)
                v.tensor_reduce(av(D32, comp * 16, [[32, KC], [1, 16], [0, 1]]),
                                av(sc0, 0, [[64, KC], [4, 16], [1, 4]]),
                                axis=AX, op=ADD)
            v.tensor_mul(av(D32, 0, [[32, KC], [1, 32]]),
                         av(D32, 0, [[32, KC], [1, 32]]),
                         av(u_t, k0, [[1, KC], [0, 32]]))
            # b4e = B4(D) [oc,r2,c,r], B[r2,r]=D[r,r2]
            for (qoff, soff, sgn) in ((0, 0, 1.0), (4, 16, -1.0),
                                      (32, 16, 1.0), (36, 0, 1.0)):
                if sgn > 0:
                    v.tensor_copy(av(b4e, qoff, [[64, KC], [8, 4], [1, 4]]),
                                  av(D32, soff, [[32, KC], [1, 4], [4, 4]]))
                else:
                    v.tensor_scalar_mul(av(b4e, qoff, [[64, KC], [8, 4], [1, 4]]),
                                        av(D32, soff, [[32, KC], [1, 4], [4, 4]]), -1.0)

            # E = T' @ D: A=a3ve, B4=b4e; d1=g,d2=r2,j=r
            for kk in range(KC):
                cmat(a3ve, (k0 + kk) * 128, 8, 4, b4e, kk * 64, 32, 8, 4,
                     Ng, Nrf, Nrf, p_off=kk * 1024)
            kt, ko = fold_reduce(Pt, 0, KC * 2 * Ng * Nrf, 8)
            v.tensor_copy(ke[:, 0:KC * 128], av(kt, ko, [[1, KC * 128]]))

            # (B3(V) for W built right after V fold, above)
            for kk in range(KC):
                cmat3(lam_t, (k0 + kk) * WLAM, 48, 16, b3w, kk * 192, 64, 16,
                      Ng, Nrf, Ng, p_off=kk * 3072)
            kt, ko = fold_reduce(Pt, 0, KC * 3 * Ng * Nrf, 16)
            # mgf = W - E into mgf_t [g,cc,r,oc]: Wre=K1-K3, Wim=K1+K2
            # (0,0)=(1,1)=mre=Wre-KE0; (0,1)=mim=Wim-KE1; (1,0)=-mim
            for kk in range(KC):
                moff = (k0 + kk) * 256
                kbw = ko + kk * 192
                kbe = kk * 128
                d9 = av(mgf_t, moff, [[9, 2], [16, 16], [2, 4]])
                v.tensor_sub(d9, av(kt, kbw, [[0, 2], [4, 16], [1, 4]]),
                             av(kt, kbw + 128, [[0, 2], [4, 16], [1, 4]]))
                v.tensor_sub(d9, d9, av(ke, kbe, [[0, 2], [4, 16], [1, 4]]))
                d1v = av(mgf_t, moff + 1, [[16, 16], [2, 4]])
                v.tensor_add(d1v, av(kt, kbw, [[4, 16], [1, 4]]),
                             av(kt, kbw + 64, [[4, 16], [1, 4]]))
                v.tensor_sub(d1v, d1v, av(ke, kbe + 64, [[4, 16], [1, 4]]))
                d8 = av(mgf_t, moff + 8, [[16, 16], [2, 4]])
                v.tensor_sub(d8, av(ke, kbe + 64, [[4, 16], [1, 4]]),
                             av(kt, kbw, [[4, 16], [1, 4]]))
                v.tensor_sub(d8, d8, av(kt, kbw + 64, [[4, 16], [1, 4]]))

        # ---------- PE: eg = sum_k Gh @ mgf ----------
        # mgf_t -> mstg (k-major), then gather stationary sta:
        # rows 32*kq+2g+cc (4 k per matmul), cols (sample*4+kgrp)*8+roc
        nc.sync.dma_start(out=mstg[h].rearrange("k p w -> p k w"),
                          in_=mgf_t.rearrange("p (k w) -> p k w", k=Nk))
        sta = blk.tile([P, 4096], F16, name="sta")
        m0 = mstg[h, 0]
        for kq in range(4):
            for kgrp in range(4):
                nc.sync.dma_start(
                    out=bass.AP(tensor=sta.tensor,
                                offset=sta.offset + 32 * kq * 4096 + kgrp * 8,
                                ap=[[4096, 32], [32, P], [1, 8]]),
                    in_=bass.AP(tensor=m0.tensor,
                                offset=m0.offset + (kgrp * 4 + kq) * P * 256,
                                ap=[[8, 32], [256, P], [1, 8]]))
        # moving: separate tile so next block's GF can overlap this tail
        g3mov = blk.tile([P, 64 * Nk * Ng], F16, name="g3mov")
        nc.sync.dma_start(out=g3mov, in_=g3[h])
        egsb = blk.tile([P, 1024], F32, name="egsb")
        for sgrp in range(32):
            pt = psum.tile([P, 32], F32, name="peg")
            for sq in range(4):
                s = sgrp * 4 + sq
                for kgrp in range(4):
                    nc.tensor.matmul(
                        out=pt[32 * sq:32 * sq + 8, :],
                        lhsT=av(sta, (s * 4 + kgrp) * 8, [[1, 8]]),
                        rhs=av(g3mov, (s * 4 + kgrp) * 32, [[1, 32]]),
                        start=(kgrp == 0), stop=(kgrp == 3),
                        tile_position=(0, 32 * sq))
            nc.scalar.activation(egsb[:, sgrp * 32:(sgrp + 1) * 32], pt, AF.Copy)
        nc.sync.dma_start(out=estg[h], in_=egsb)
        # gather eg to sample-major egT [p, (c,n,r)]: per (oc, s4)
        e0 = estg[h]
        for oc in (0, 1):
            for s4 in range(4):
                nc.sync.dma_start(
                    out=bass.AP(tensor=egT.tensor,
                                offset=egT.offset + s4 * 256 + oc * 128,
                                ap=[[1024, 32], [4, 32], [1, 4]]),
                    in_=bass.AP(tensor=e0.tensor,
                                offset=e0.offset + (32 * s4 + oc) * 1024,
                                ap=[[32, 32], [1, 32], [2048, 4]]))

        # ---------- epilogue ----------
        if dbg is not None:
            nc.sync.dma_start(out=dbg["egT"][hs:hs + P], in_=egT)
        egr = egT[:, 0:128]
        egi = egT[:, 128:256]
        v.tensor_scalar_mul(egr, egr, -1.0 / Nk)
        v.tensor_scalar_mul(egi, egi, -1.0 / Nk)
        v.tensor_mul(egr, egr, pm_t)
        v.tensor_mul(egi, egi, pm_t)

        # step MLP (fixed leaky-relu)
        iq = blk.tile([P, 64], F32, name="iq")
        v.tensor_reduce(av(iq, 0, [[1, 32], [0, 1]]),
                        av(f32_t, 0, [[4, 32], [1, 4]]), axis=AX, op=ADD)
        v.tensor_reduce(av(iq, 32, [[1, 32], [0, 1]]),
                        av(f32_t, 128, [[4, 32], [1, 4]]), axis=AX, op=ADD)
        v.tensor_mul(iq, iq, bnsc)
        v.tensor_add(iq, iq, bnsh)
        v.tensor_mul(iq, iq, dw_t)
        z = blk.tile([P, 1], F32, name="z")
        v.tensor_reduce(z, iq.unsqueeze(1), axis=AX, op=ADD)
        v.tensor_add(z, z, db_t)
        smax = blk.tile([P, 1], F32, name="smax")
        step = blk.tile([P, 1], F32, name="step")
        v.tensor_scalar_max(smax, z, 0.0)
        v.tensor_scalar_min(step, z, 0.0)
        v.scalar_tensor_tensor(out=step, in0=step, scalar=ALPHA, in1=smax,
                               op0=MULT, op1=ADD)

        # proj = Re(eg * conj(Frf)); rg = eg - proj*Frf
        proj = blk.tile([P, 128], F32, name="proj")
        t0 = sc1[:, 0:128]
        t1 = sc2[:, 0:128]
        v.tensor_mul(proj, egr, fre)
        v.tensor_mul(t0, egi, fim)
        v.tensor_add(proj, proj, t0)
        rgr = blk.tile([P, 128], F32, name="rgr")
        rgi = blk.tile([P, 128], F32, name="rgi")
        v.tensor_mul(t0, proj, fre)
        v.tensor_sub(rgr, egr, t0)
        v.tensor_mul(t0, proj, fim)
        v.tensor_sub(rgi, egi, t0)

        # nrm; sc = -step/nrm
        n2 = blk.tile([P, 1], F32, name="n2")
        v.tensor_mul(t0, rgr, rgr)
        v.tensor_mul(t1, rgi, rgi)
        v.tensor_add(t0, t0, t1)
        v.tensor_reduce(n2, t0.unsqueeze(1), axis=AX, op=ADD)
        nc.scalar.activation(n2, n2, AF.Sqrt, bias=zero1)
        v.reciprocal(n2, n2)
        v.tensor_mul(n2, n2, step)
        v.tensor_scalar_mul(n2, n2, -1.0)
        fnr = blk.tile([P, 128], F32, name="fnr")
        fni = blk.tile([P, 128], F32, name="fni")
        v.scalar_tensor_tensor(out=fnr, in0=rgr, scalar=n2, in1=fre,
                               op0=MULT, op1=ADD)
        v.scalar_tensor_tensor(out=fni, in0=rgi, scalar=n2, in1=fim,
                               op0=MULT, op1=ADD)

        # scale = relu(|fnew|-1)+1 ; out = fnew/scale (interleaved)
        m2 = blk.tile([P, 128], F32, name="m2")
        v.tensor_mul(m2, fnr, fnr)
        v.tensor_mul(t0, fni, fni)
        v.tensor_add(m2, m2, t0)
        nc.scalar.activation(m2, m2, AF.Sqrt, bias=zero1)
        nc.scalar.activation(m2, m2, AF.Relu, bias=neg1)
        v.tensor_scalar_add(m2, m2, 1.0)
        v.reciprocal(m2, m2)
        ob = blk.tile([P, 256], F32, name="ob")
        v.tensor_mul(av(ob, 0, [[2, 128]]), fnr, m2)
        v.tensor_mul(av(ob, 1, [[2, 128]]), fni, m2)
        nc.sync.dma_start(out=out_v[hs:hs + P], in_=ob)


def inv4_batched(v, scr, S32, Y32):
    """Y32 = inv(S32) for Nk batched 4x4 complex mats, layout [k,c,i,j] f32.
    2x2 block Schur: S=[[A,B],[C,D]] -> iA, E=D-C iA B, iE, assemble."""
    KS = 32  # per-k stride

    # mini complex 2x2 tiles: layout [k, c(4), i(2), j(1)] width Nk*8
    mt = {nm: scr.tile([P, Nk * 8], F32, name="m_" + nm)
          for nm in ("iA", "iE", "P2", "Q2", "T2", "E2")}
    sc = scr.tile([P, Nk * 16], F32, name="m_sc")

    def sl(i):  # scratch slot [p, Nk]
        return av(sc, i * Nk, [[1, Nk]])

    def ent(t, base, c, i, j, mini=False):
        if mini:
            return av(t, c * 4 + i * 2 + j, [[8, Nk]])
        return av(t, base + c * 16 + i * 4 + j, [[KS, Nk]])

    def blkv(t, base, c, mini=False):
        if mini:
            return av(t, c * 4, [[8, Nk], [2, 2], [1, 2]])
        return av(t, base + c * 16, [[KS, Nk], [4, 2], [1, 2]])

    def c2inv(dst, src, sbase, smini):
        """dst (mini) = inv of 2x2 complex block of src at sbase."""
        e = lambda c, i, j: ent(src, sbase, c, i, j, smini)
        dre, dim, q0, q1 = sl(0), sl(1), sl(2), sl(3)
        # det = s00*s11 - s01*s10
        v.tensor_mul(dre, e(0, 0, 0), e(0, 1, 1))
        v.tensor_mul(q0, e(1, 0, 0), e(1, 1, 1))
        v.tensor_sub(dre, dre, q0)
        v.tensor_mul(q0, e(0, 0, 1), e(0, 1, 0))
        v.tensor_sub(dre, dre, q0)
        v.tensor_mul(q0, e(1, 0, 1), e(1, 1, 0))
        v.tensor_add(dre, dre, q0)
        v.tensor_mul(dim, e(0, 0, 0), e(1, 1, 1))
        v.tensor_mul(q0, e(1, 0, 0), e(0, 1, 1))
        v.tensor_add(dim, dim, q0)
        v.tensor_mul(q0, e(0, 0, 1), e(1, 1, 0))
        v.tensor_sub(dim, dim, q0)
        v.tensor_mul(q0, e(1, 0, 1), e(0, 1, 0))
        v.tensor_sub(dim, dim, q0)
        # inv det (conj form)
        v.tensor_mul(q0, dre, dre)
        v.tensor_mul(q1, dim, dim)
        v.tensor_add(q0, q0, q1)
        v.reciprocal(q0, q0)
        v.tensor_mul(dre, dre, q0)
        v.tensor_mul(dim, dim, q0)
        v.tensor_scalar_mul(dim, dim, -1.0)
        # adj entries into sc slots 4..11: [a11,-a01,-a10,a00] per comp
        for c in (0, 1):
            base_sl = 4 + c * 4
            v.tensor_copy(sl(base_sl + 0), e(c, 1, 1))
            v.tensor_scalar_mul(sl(base_sl + 1), e(c, 0, 1), -1.0)
            v.tensor_scalar_mul(sl(base_sl + 2), e(c, 1, 0), -1.0)
            v.tensor_copy(sl(base_sl + 3), e(c, 0, 0))
        adjr = av(sc, 4 * Nk, [[1, Nk], [2 * Nk, 2], [Nk, 2]])
        adji = av(sc, 8 * Nk, [[1, Nk], [2 * Nk, 2], [Nk, 2]])
        q2 = av(sc, 12 * Nk, [[1, Nk], [2 * Nk, 2], [Nk, 2]])
        dreb = av(sc, 0, [[1, Nk], [0, 2], [0, 2]])
        dimb = av(sc, Nk, [[1, Nk], [0, 2], [0, 2]])
        dr = blkv(dst, 0, 0, True)
        di = blkv(dst, 0, 1, True)
        # re = adjr*dre - adji*dim ; im = adjr*dim + adji*dre
        v.tensor_mul(q2, adjr, dreb)
        v.tensor_mul(dr, adji, dimb)
        v.tensor_sub(dr, q2, dr)
        v.tensor_mul(q2, adjr, dimb)
        v.tensor_mul(di, adji, dreb)
        v.tensor_add(di, q2, di)

    def c2mul(dst, x, xbase, xmini, y, ybase, ymini,
              sub_from=None, sf_base=0, sf_mini=False, negate=False):
        """dst (mini) = x@y | sub_from - x@y | -(x@y), 2x2 complex."""
        t0f = av(sc, 0, [[1, Nk], [Nk, 8]])
        t1f = av(sc, 8 * Nk, [[1, Nk], [Nk, 8]])

        def bxr(t, base, c, i, mini):
            # row i of x block: [k, j(bcast), m]
            if mini:
                return av(t, c * 4 + i * 2, [[8, Nk], [0, 2], [1, 2]])
            return av(t, base + c * 16 + i * 4, [[KS, Nk], [0, 2], [1, 2]])

        def by(t, base, c, mini):
            # y block as [k, j, m] with entry (m,j)
            if mini:
                return av(t, c * 4, [[8, Nk], [1, 2], [2, 2]])
            return av(t, base + c * 16, [[KS, Nk], [1, 2], [4, 2]])

        for comp in (0, 1):
            for i in (0, 1):
                t0r = av(sc, i * 4 * Nk, [[1, Nk], [2 * Nk, 2], [Nk, 2]])
                t1r = av(sc, (8 + i * 4) * Nk, [[1, Nk], [2 * Nk, 2], [Nk, 2]])
                v.tensor_mul(t0r, bxr(x, xbase, 0, i, xmini), by(y, ybase, comp, ymini))
                v.tensor_mul(t1r, bxr(x, xbase, 1, i, xmini), by(y, ybase, 1 - comp, ymini))
            if comp == 0:
                v.tensor_sub(t0f, t0f, t1f)
            else:
                v.tensor_add(t0f, t0f, t1f)
            d = blkv(dst, 0, comp, True)
            red = av(sc, 8 * Nk, [[1, Nk], [Nk, 4], [0, 1]])
            v.tensor_reduce(red, av(sc, 0, [[1, Nk], [2 * Nk, 4], [Nk, 2]]),
                            axis=AX, op=ADD)
            redv = av(sc, 8 * Nk, [[1, Nk], [2 * Nk, 2], [Nk, 2]])
            if sub_from is not None:
                v.tensor_sub(d, blkv(sub_from, sf_base, comp, sf_mini), redv)
            elif negate:
                v.tensor_scalar_mul(d, redv, -1.0)
            else:
                v.tensor_copy(d, redv)

    iA, iE, P2, Q2, T2, E2 = (mt[n] for n in ("iA", "iE", "P2", "Q2", "T2", "E2"))
    # S blocks: A off 0, B off 2, C off 8, D off 10
    c2inv(iA, S32, 0, False)
    c2mul(P2, iA, 0, True, S32, 2, False)                 # P2 = iA@B
    c2mul(Q2, S32, 8, False, iA, 0, True)                 # Q2 = C@iA
    c2mul(E2, S32, 8, False, P2, 0, True,
          sub_from=S32, sf_base=10, sf_mini=False)        # E2 = D - C@P2
    c2inv(iE, E2, 0, True)
    c2mul(T2, P2, 0, True, iE, 0, True, negate=True)      # T2 = -P2@iE = Y12
    # write Y12 -> Y32[0:2,2:4]
    for c in (0, 1):
        v.tensor_copy(blkv(Y32, 2, c), blkv(T2, 0, c, True))
    c2mul(E2, iE, 0, True, Q2, 0, True, negate=True)      # E2 = -iE@Q2 = Y21
    for c in (0, 1):
        v.tensor_copy(blkv(Y32, 8, c), blkv(E2, 0, c, True))
    # Y11 = iA - Y12@Q2  (T2 holds Y12; write to E2 to avoid dst/src alias)
    c2mul(E2, T2, 0, True, Q2, 0, True,
          sub_from=iA, sf_base=0, sf_mini=True)
    for c in (0, 1):
        v.tensor_copy(blkv(Y32, 0, c), blkv(E2, 0, c, True))
    for c in (0, 1):
        v.tensor_copy(blkv(Y32, 10, c), blkv(iE, 0, c, True))


# ================= host side =================

_NC_CACHE = {}


def _prep(inputs):
    f16, f32 = np.float16, np.float32
    Gr = np.asarray(inputs["G_re"], f32)
    Gi = np.asarray(inputs["G_im"], f32)
    Lr = np.asarray(inputs["Lam_re"], f32)
    Li = np.asarray(inputs["Lam_im"], f32)
    fr = np.asarray(inputs["Frf_re"], f32)
    fi = np.asarray(inputs["Frf_im"], f32)


    lamx = np.empty((Nk, B, Ng, 3, Ng), f16)    # Lam 3-slot: [g,s,g']
    lamx[..., 0, :] = Lr + Li
    lamx[..., 1, :] = Lr
    lamx[..., 2, :] = Li
    frT = fr.transpose(0, 2, 1)                  # [B,r,n]
    fiT = fi.transpose(0, 2, 1)
    fb4 = np.empty((B, 2, Nrf, 2, Nt), f16)     # B4(Frf): [oc,r,c,n]
    fb4[:, 0, :, 0, :] = frT
    fb4[:, 0, :, 1, :] = -fiT
    fb4[:, 1, :, 0, :] = fiT
    fb4[:, 1, :, 1, :] = frT
    f32x = np.empty((B, 2, Nt, Nrf), f32)
    f32x[:, 0] = fr
    f32x[:, 1] = fi
    u32 = np.ascontiguousarray(
        (1.0 / np.asarray(inputs["Beta_re"], f32).reshape(Nk, B).T))
    aux = np.concatenate([
        np.asarray(inputs["bn_gamma"], f32).ravel(),
        np.asarray(inputs["bn_beta"], f32).ravel(),
        np.asarray(inputs["bn_mean"], f32).ravel(),
        np.asarray(inputs["bn_var"], f32).ravel(),
        np.asarray(inputs["dense_w"], f32).ravel(),
        np.asarray(inputs["dense_b"], f32).ravel(),
        np.asarray(inputs["P_mask"], f32).ravel()])

    in_maps = []
    for c in range(NCORES):
        s = slice(c * BL, (c + 1) * BL)
        # gpe: moving G for PE-GF: [h, row=64s+32cc+n, pair*256 + k*16+g]
        Xc = np.stack([Gr[:, s], Gi[:, s]])          # [cc,k,BL,n,g]
        Xc = Xc.reshape(2, Nk, H, 64, 2, Nt, Ng)     # [cc,k,h,q,ss,n,g]
        gpe = np.ascontiguousarray(
            Xc.transpose(2, 4, 0, 5, 3, 1, 6).reshape(H, P, 64 * Nk * Ng)
        ).astype(f16)
        # fpe: block-diag stationary: rows as gpe, cols pair*16 + 8s+2r+oc
        # oc=0 (re): (cc0: fr, cc1: fi) ; oc=1 (-im, conj): (cc0: -fi, cc1: fr)
        frc = fr[s].reshape(H, 64, 2, Nt, Nrf)       # [h,q,ss,n,r]
        fic = fi[s].reshape(H, 64, 2, Nt, Nrf)
        fpe_a = np.zeros((H, 2, 2, Nt, 64, 16), np.float32)  # [h,ss,cc,n,q,col]
        for ss in (0, 1):
            fq = frc[:, :, ss].transpose(0, 2, 1, 3)  # [h,n,q,r]
            gq = fic[:, :, ss].transpose(0, 2, 1, 3)
            cb = np.empty((2, H, Nt, 64, Nrf, 2), np.float32)  # [cc,h,n,q,r,oc]
            cb[0, ..., 0] = fq
            cb[0, ..., 1] = -gq
            cb[1, ..., 0] = gq
            cb[1, ..., 1] = fq
            for cc in (0, 1):
                fpe_a[:, ss, cc, :, :, 8 * ss:8 * ss + 8] = \
                    cb[cc].reshape(H, Nt, 64, 8)
        fpe = np.ascontiguousarray(fpe_a.reshape(H, P, 64 * 16)).astype(f16)
        # g3: eg moving operand: rows 32*kq+2g+cc, cols (sample*4+kgrp)*32+n
        Yc = np.stack([Gr[:, s], Gi[:, s]])          # [cc,k,BL,n,g]
        Yc = Yc.reshape(2, 4, 4, H, P, Nt, Ng)       # [cc,kgrp,kq,h,smp,n,g]
        g3 = np.ascontiguousarray(
            Yc.transpose(3, 2, 6, 0, 4, 1, 5).reshape(H, P, 32 * Nk * Nt)
        ).astype(f16)
        in_maps.append({
            "gpe": gpe,
            "fpe": fpe,
            "g3": g3,
            "lam": np.ascontiguousarray(lamx[:, s]).reshape(Nk, BL, WLAM),
            "fb4": np.ascontiguousarray(fb4[s]).reshape(BL, 512),
            "f32t": np.ascontiguousarray(f32x[s]).reshape(BL, 256),
            "u32": np.ascontiguousarray(u32[s]),
            "aux": aux,
        })
    return in_maps


def kernel(**inputs):
    in_maps = _prep(inputs)
    if "nc" not in _NC_CACHE:
        _NC_CACHE["nc"] = build_nc()
    nc = _NC_CACHE["nc"]
    from concourse.bass_utils import run_bass_kernel_spmd
    res = run_bass_kernel_spmd(nc, in_maps, core_ids=list(range(NCORES)))
    outs = [res.results[c]["out"] for c in range(NCORES)]
    full = np.concatenate(outs, axis=0).reshape(B, Nt, Nrf, 2)
    return np.ascontiguousarray(full).view(np.complex64).reshape(B, Nt, Nrf)


def kernel_profiled(**inputs):
    """Returns modeled HW exec time (ns) from the cost-model timeline sim."""
    if "nc" not in _NC_CACHE:
        _NC_CACHE["nc"] = build_nc()
    from concourse.timeline_sim import TimelineSim
    ts = TimelineSim(_NC_CACHE["nc"], no_exec=True)
    return int(ts.simulate())


# revision 62
# speedup vs baseline: 1.0664x; 1.0040x over previous
"""Trainium2 Bass kernel for nn_DUP_block_90391881712206.

Math per (k,b) via Woodbury (no 16x16 inversions):
    GF = conj(Gh)^T @ Frf            [Ng,Nrf]
    T' = Lam @ GF
    S  = I + u * GF^H @ T'  (u = 1/beta)
    Y  = inv(S)  (4x4 complex, 2x2 block Schur, f32)
    V  = u * T' @ Y
    C  = T'^H @ V ; D = u * Y @ C
    E  = T' @ D ; W = Lam @ V ; mgf = W - E
    eg_k = Gh @ mgf
eg = -mean_k(eg_k) * P_mask, then Riemannian step + unit-disk clamp.

Implementation: data-parallel over B across 8 cores (256 b/core, 2 blocks
of 128 partitions). Batched small complex matmuls on DVE in fp16: one
broadcast-AP multiply builds the product tensor [outc,d1,d2,c,j] (packed
last dim -> 2x DVE mode), then log2 halving-fold adds reduce segments
(packed -> 2x). Host pre-lays inputs (conj baked into G's imag for GF).
"""

import numpy as np
import sys
from contextlib import ExitStack

sys.path.insert(0, "/opt/trn_rl_repo")

import concourse.bass as bass
import concourse.bacc as bacc_mod
import concourse.tile as tile
from concourse import mybir

Nk, B, Nt, Nrf, Ng = 16, 2048, 32, 4, 16
NCORES = 8
BL = B // NCORES
P = 128
H = BL // P
KC = 2                     # k per chunk
NCH = Nk // KC
BN_EPS = 1e-3
ALPHA = 0.1

F16 = mybir.dt.float16
F32 = mybir.dt.float32
AX = mybir.AxisListType.X
ADD = mybir.AluOpType.add
SUB = mybir.AluOpType.subtract
MULT = mybir.AluOpType.mult
AF = mybir.ActivationFunctionType

# per-k widths (fp16 elems)
WG1 = Ng * 2 * Nt          # 1024  g1 [g,c,n]
WG2 = Nt * 2 * Ng          # 1024  g2 [n,c,g]
WLAM = Ng * 3 * Ng         # 768   lam 3-slot [g,s,g'] (Karatsuba)


def av(t, off, dims):
    """AP view of tile t at free-offset `off` with free dims [[stride,n],..]."""
    return bass.AP(tensor=t.tensor, offset=t.offset + off,
                   ap=[list(t.ap[0])] + [list(d) for d in dims])


def build_nc(debug=False):
    nc = bacc_mod.Bacc()
    gpe = nc.dram_tensor("gpe", [H, P, 64 * Nk * Ng], F16, kind="ExternalInput")
    fpe = nc.dram_tensor("fpe", [H, P, 64 * 16], F16, kind="ExternalInput")
    stg = nc.dram_tensor("stg", [H, 16, P, 256], F16, kind="Internal")
    g3 = nc.dram_tensor("g3", [H, P, 32 * Nk * Nt], F16, kind="ExternalInput")
    mstg = nc.dram_tensor("mstg", [H, Nk, P, 256], F16, kind="Internal")
    estg = nc.dram_tensor("estg", [H, P, 1024], F32, kind="Internal")
    lam = nc.dram_tensor("lam", [Nk, BL, WLAM], F16, kind="ExternalInput")
    fb4 = nc.dram_tensor("fb4", [BL, 2 * Nrf * 2 * Nt], F16, kind="ExternalInput")
    f32t = nc.dram_tensor("f32t", [BL, 2 * Nt * Nrf], F32, kind="ExternalInput")
    u32 = nc.dram_tensor("u32", [BL, Nk], F32, kind="ExternalInput")
    aux = nc.dram_tensor("aux", [5 * 2 * Nt + 1 + Nt * Nrf], F32, kind="ExternalInput")
    out = nc.dram_tensor("out", [BL, Nt * Nrf * 2], F32, kind="ExternalOutput")
    dbg = None
    if debug:
        dbg = {nm: nc.dram_tensor("dbg_" + nm, [BL, w], dt, kind="ExternalOutput")
               for nm, w, dt in (("a3s", Nk * 128, F16), ("a3ve", Nk * 128, F16),
                                 ("S32", Nk * 32, F32), ("Y32", Nk * 32, F32),
                                 ("vw", Nk * 256, F16), ("keg", Nk * 256, F16),
                                 ("egT", 256, F32))}

    with ExitStack() as ctx:
        tc = ctx.enter_context(tile.TileContext(nc))
        kern(ctx, tc, gpe, fpe, stg, g3, mstg, estg, lam, fb4, f32t, u32, aux,
             out, dbg)
    if not nc.is_finalized():
        nc.finalize()
    return nc


def kern(ctx, tc, gpe, fpe, stg, g3, mstg, estg, lam, fb4, f32t, u32, aux,
         out, dbg=None):
    nc = tc.nc
    v = nc.vector

    singles = ctx.enter_context(tc.tile_pool(name="singles", bufs=1))
    loads = ctx.enter_context(tc.tile_pool(name="loads", bufs=3))
    blk = ctx.enter_context(tc.tile_pool(name="blk", bufs=1))
    scr = ctx.enter_context(tc.tile_pool(name="scr", bufs=1))
    psum = ctx.enter_context(tc.tile_pool(name="psum", bufs=4, space="PSUM"))

    # ---- broadcast-load aux params ----
    aux_t = singles.tile([P, 449], F32, name="aux_bc")
    aux_ap = aux[:]
    src = bass.AP(tensor=aux_ap.tensor, offset=aux_ap.offset,
                  ap=[[0, P]] + list(aux_ap.ap))
    nc.gpsimd.dma_start(out=aux_t, in_=src)
    gam_t, bb_t = aux_t[:, 0:64], aux_t[:, 64:128]
    bm_t, bv_t = aux_t[:, 128:192], aux_t[:, 192:256]
    dw_t, db_t = aux_t[:, 256:320], aux_t[:, 320:321]
    pm_t = aux_t[:, 321:449]

    zero1 = singles.tile([P, 1], F32, name="zero1")
    neg1 = singles.tile([P, 1], F32, name="neg1")
    v.memset(zero1, 0.0)
    v.memset(neg1, -1.0)
    eye16 = singles.tile([P, 16], F32, name="eye16")
    v.memset(eye16, 0.0)
    v.memset(av(eye16, 0, [[5, 4]]), 1.0)

    # bn scale/shift
    bnsc = singles.tile([P, 64], F32, name="bnsc")
    bnsh = singles.tile([P, 64], F32, name="bnsh")
    v.tensor_scalar_add(bnsc, bv_t, BN_EPS)
    v.reciprocal(bnsc, bnsc)
    nc.scalar.activation(bnsc, bnsc, AF.Sqrt, bias=zero1)
    v.tensor_mul(bnsc, bnsc, gam_t)
    v.tensor_mul(bnsh, bm_t, bnsc)
    v.tensor_sub(bnsh, bb_t, bnsh)

    # ---- scratch tiles ----
    # product tensors + fold ping-pong (sized for biggest cmat: 4w=8192/k)
    Pt = scr.tile([P, KC * 4096], F16, name="Pt")
    Pf1 = scr.tile([P, KC * 2048], F16, name="Pf1")
    Pf2 = scr.tile([P, KC * 1024], F16, name="Pf2")
    sc0 = scr.tile([P, KC * 256], F32, name="sc0")   # f32 scratch (D etc)
    sc1 = scr.tile([P, 256], F32, name="sc1")
    sc2 = scr.tile([P, 256], F32, name="sc2")

    def fold_reduce(src_t, src_off, nseg, L, eng1=None):
        """Sum contiguous segments of length L (pow2) via halving adds.
        Returns (tile, offset) of compact [nseg] result. eng1: engine for
        the first (widest) fold level (e.g. nc.gpsimd to offload)."""
        cur_t, cur_off, cl = src_t, src_off, L
        ping = [Pf1, Pf2]
        pi = 0
        first = True
        while cl > 1:
            half = cl // 2
            dst = ping[pi]
            pi ^= 1
            eng = eng1 if (first and eng1 is not None) else v
            eng.tensor_add(av(dst, 0, [[half, nseg], [1, half]]),
                           av(cur_t, cur_off, [[cl, nseg], [1, half]]),
                           av(cur_t, cur_off + half, [[cl, nseg], [1, half]]))
            cur_t, cur_off, cl = dst, 0, half
            first = False
        return cur_t, cur_off

    def cmat(a_t, a_off, a_d1s, a_cs, b4_t, b4_off, b4_ocs, b4_d2s, b4_cs,
             d1, d2, j, p_off=0):
        """Product P[outc,d1,d2,c,j] = A[d1,c,j] * B4[outc,d2,c,j], then
        fold-reduce (c,j) -> K [outc,d1,d2] compact fp16. One k at a time
        for the mul (4-dim APs); returns nothing (call fold separately)."""
        L = 2 * j
        w2 = d1 * d2 * L
        a_v = av(a_t, a_off, [[a_d1s, d1], [0, d2], [1, L]])
        for oc in range(2):
            ov = av(Pt, p_off + oc * w2, [[d2 * L, d1], [L, d2], [1, L]])
            b_v = av(b4_t, b4_off + oc * b4_ocs, [[0, d1], [b4_d2s, d2], [1, L]])
            v.tensor_mul(ov, a_v, b_v)

    def cmat3(a_t, a_off, a_d1s, a_ss, b3_t, b3_off, b3_ss, b3_d2s,
              d1, d2, j, p_off=0):
        """Gauss 3-mult: P3[s,d1,d2,j] = A3[d1,s,j] * B3[s,d2,j];
        fold j -> K [s,d1,d2]; re=K1-K3, im=K1+K2."""
        w1 = d1 * d2 * j
        for sl in range(3):
            ov = av(Pt, p_off + sl * w1, [[d2 * j, d1], [j, d2], [1, j]])
            a_v = av(a_t, a_off + sl * a_ss, [[a_d1s, d1], [0, d2], [1, j]])
            b_v = av(b3_t, b3_off + sl * b3_ss, [[0, d1], [b3_d2s, d2], [1, j]])
            v.tensor_mul(ov, a_v, b_v)

    # ---------- PE: GF heads for both blocks (hoisted for overlap) ----------
    # stationary fpe [128=(s,cc,n), 16=(s,r,oc)] per pair (block-diag Frf);
    # moving gmov [128, 256=(k,g)] per pair; out psum [16, 256] per pair.
    gmov = blk.tile([P, 64 * Nk * Ng], F16, name="gmov")
    fpe_t = blk.tile([P, 64 * 16], F16, name="fpe_t")

    def gf_head(hh):
        nc.sync.dma_start(out=gmov, in_=gpe[hh])
        nc.sync.dma_start(out=fpe_t, in_=fpe[hh])
        for grp in range(16):
            pt = psum.tile([P, 256], F32, name="pgf")
            for qq in range(4):
                pair = grp * 4 + qq
                nc.tensor.matmul(out=pt[32 * qq:32 * qq + 16, :],
                                 lhsT=fpe_t[:, pair * 16:(pair + 1) * 16],
                                 rhs=av(gmov, pair * 256, [[1, 256]]),
                                 start=True, stop=True,
                                 tile_position=(0, 32 * qq))
            sgf = loads.tile([P, 256], F16, name="sgf")
            nc.scalar.activation(sgf, pt, AF.Copy)
            nc.sync.dma_start(out=stg[hh, grp], in_=sgf)

    gf_head(0)

    # ============ per-block main ============
    out_v = out
    for h in range(H):
        hs = h * P

        fb4_t = blk.tile([P, 512], F16, name="fb4_t")
        f32_t = blk.tile([P, 256], F32, name="f32_t")
        u_t = blk.tile([P, Nk], F32, name="u_t")
        nc.sync.dma_start(out=fb4_t, in_=fb4[hs:hs + P])
        nc.sync.dma_start(out=f32_t, in_=f32t[hs:hs + P])
        nc.sync.dma_start(out=u_t, in_=u32[hs:hs + P])
        fre = f32_t[:, 0:128]
        fim = f32_t[:, 128:256]

        lam_t = blk.tile([P, Nk * WLAM], F16, name="lam_t")
        nc.sync.dma_start(out=lam_t.rearrange("p (k w) -> p k w", k=Nk),
                          in_=lam[:, hs:hs + P].rearrange("k b w -> b k w"))

        # all-k mid tensors (fp16 per-k layouts)
        a3ve = blk.tile([P, Nk * 128], F16, name="a3ve")  # T' [g,c,r]
        a3c = blk.tile([P, Nk * 128], F16, name="a3c")    # conj(T') [r,c,g']
        b4s = blk.tile([P, Nk * 256], F16, name="b4s")    # B4(T') [oc,r2,c,g']
        a3s = blk.tile([P, Nk * 128], F16, name="a3s")    # conj(GF) [r,c,g']
        b3t = blk.tile([P, Nk * 192], F16, name="b3t")    # B3(GF) [s,r,g']
        b3w = blk.tile([P, KC * 192], F16, name="b3w")    # B3(V) [s,r2,g']
        S32 = blk.tile([P, Nk * 32], F32, name="S32")     # [k,c,i,j]
        Y32 = blk.tile([P, Nk * 32], F32, name="Y32")
        Yu32 = blk.tile([P, Nk * 32], F32, name="Yu32")
        b4y = blk.tile([P, Nk * 64], F16, name="b4y")     # B4(Yu) [oc,r2,c,r]
        b4cw = blk.tile([P, KC * 256], F16, name="b4cw")  # B4(V) [oc,r2,c,g']
        C32 = blk.tile([P, KC * 32], F32, name="C32")
        D32 = blk.tile([P, KC * 32], F32, name="D32")
        b4e = blk.tile([P, KC * 64], F16, name="b4e")     # B4(D) [oc,r2,c,r]
        ke = blk.tile([P, KC * 128], F16, name="ke")      # saved K of E
        egT = blk.tile([P, 256], F32, name="egT")
        mgf_t = blk.tile([P, Nk * 256], F16, name="mgf_t")

        # (GF PE head hoisted before the block loop)
        # stg row (per grp): 32*qq + 8s + 2r + oc ; col = k*16+g.
        # gather to a3s [p=sample, (r,c,k,g')]: 1 DMA per s
        st0 = stg[h, 0]
        pstride = Nk * 128
        for s in (0, 1):
            for r in range(4):
                for c in range(2):
                    nc.sync.dma_start(
                        out=bass.AP(tensor=a3s.tensor,
                                    offset=a3s.offset + s * pstride + r * 32 + c * 16,
                                    ap=[[2 * pstride, 64], [128, 16], [1, 16]]),
                        in_=bass.AP(tensor=st0.tensor,
                                    offset=st0.offset + s * 2048 + r * 512 + c * 256,
                                    ap=[[8192, 64], [16, 16], [1, 16]]))

        # b3t = B3(GF) slots (Gre, Gim-Gre, Gre+Gim); a3s = (Gre, -Gim)
        v.tensor_copy(av(b3t, 0, [[192, Nk], [16, 4], [1, 16]]),
                      av(a3s, 0, [[128, Nk], [32, 4], [1, 16]]))
        v.scalar_tensor_tensor(
            out=av(b3t, 64, [[192, Nk], [16, 4], [1, 16]]),
            in0=av(a3s, 16, [[128, Nk], [32, 4], [1, 16]]), scalar=-1.0,
            in1=av(a3s, 0, [[128, Nk], [32, 4], [1, 16]]),
            op0=MULT, op1=SUB)
        v.tensor_sub(av(b3t, 128, [[192, Nk], [16, 4], [1, 16]]),
                     av(a3s, 0, [[128, Nk], [32, 4], [1, 16]]),
                     av(a3s, 16, [[128, Nk], [32, 4], [1, 16]]))

        # ---------- pass 1: T', S per k-chunk ----------
        for ch in range(NCH):
            k0 = ch * KC

            # T' = Lam @ GF (Gauss): A3=lam [g,s,g'], B3=b3t; d1=g,d2=r,j=g'
            for kk in range(KC):
                cmat3(lam_t, (k0 + kk) * WLAM, 48, 16, b3t, (k0 + kk) * 192, 64, 16,
                      Ng, Nrf, Ng, p_off=kk * 3072)
            kt, ko = fold_reduce(Pt, 0, KC * 3 * Ng * Nrf, 16)
            # K [kk, s, g, r] strides 192,64,4,1 ; re=K1-K3, im=K1+K2
            kg0 = k0
            K1 = av(kt, ko, [[192, KC], [4, 16], [1, 4]])
            K2 = av(kt, ko + 64, [[192, KC], [4, 16], [1, 4]])
            K3 = av(kt, ko + 128, [[192, KC], [4, 16], [1, 4]])
            K1t = av(kt, ko, [[192, KC], [1, 4], [4, 16]])
            K2t = av(kt, ko + 64, [[192, KC], [1, 4], [4, 16]])
            K3t = av(kt, ko + 128, [[192, KC], [1, 4], [4, 16]])
            # a3ve = T' [g,c,r]
            v.tensor_sub(av(a3ve, kg0 * 128, [[128, KC], [8, 16], [1, 4]]), K1, K3)
            v.tensor_add(av(a3ve, kg0 * 128 + 4, [[128, KC], [8, 16], [1, 4]]), K1, K2)
            # a3c = conj(T') [r,c,g']
            v.tensor_sub(av(a3c, kg0 * 128, [[128, KC], [32, 4], [1, 16]]), K1t, K3t)
            a3c_im = av(a3c, kg0 * 128 + 16, [[128, KC], [32, 4], [1, 16]])
            v.tensor_add(a3c_im, K1t, K2t)
            v.tensor_scalar_mul(a3c_im, a3c_im, -1.0)
            # b4s = B4(T') [oc,r2,c,g']
            v.tensor_sub(av(b4s, kg0 * 256, [[256, KC], [32, 4], [1, 16]]), K1t, K3t)
            v.tensor_copy(av(b4s, kg0 * 256 + 16, [[256, KC], [32, 4], [1, 16]]),
                          a3c_im)
            v.tensor_add(av(b4s, kg0 * 256 + 128, [[256, KC], [32, 4], [1, 16]]), K1t, K2t)
            v.tensor_sub(av(b4s, kg0 * 256 + 144, [[256, KC], [32, 4], [1, 16]]), K1t, K3t)

            # S' = GF^H T': A=a3s [r1,c,g'], B4=b4s; d1=r1,d2=r2,j=g'
            for kk in range(KC):
                cmat(a3s, (k0 + kk) * 128, 32, 16, b4s, (k0 + kk) * 256, 128, 32, 16,
                     Nrf, Nrf, Ng, p_off=kk * 1024)
            kt, ko = fold_reduce(Pt, 0, KC * 2 * Nrf * Nrf, 32)
            # K [kk, outc, r1, r2] strides 32,16,4,1 -> S32 [k,c,i,j]
            v.tensor_copy(av(S32, k0 * 32, [[32, KC], [16, 2], [1, 16]]),
                          av(kt, ko, [[32, KC], [16, 2], [1, 16]]))
            # S = I + u*S'
            v.tensor_mul(av(S32, k0 * 32, [[32, KC], [1, 32]]),
                         av(S32, k0 * 32, [[32, KC], [1, 32]]),
                         av(u_t, k0, [[1, KC], [0, 32]]))
            v.tensor_add(av(S32, k0 * 32, [[32, KC], [1, 16]]),
                         av(S32, k0 * 32, [[32, KC], [1, 16]]),
                         av(eye16, 0, [[0, KC], [1, 16]]))

        if dbg is not None:
            nc.sync.dma_start(out=dbg["a3s"][hs:hs + P], in_=a3s)
            nc.sync.dma_start(out=dbg["a3ve"][hs:hs + P], in_=a3ve)
            nc.sync.dma_start(out=dbg["S32"][hs:hs + P], in_=S32)

        # ---------- pass 2: batched 4x4 inversion (2x2 Schur), f32 ----------
        if h == 0:
            gf_head(1)
        inv4_batched(v, scr, S32, Y32)
        # Yu = u*Y
        v.tensor_mul(av(Yu32, 0, [[32, Nk], [1, 32]]),
                     av(Y32, 0, [[32, Nk], [1, 32]]),
                     av(u_t, 0, [[1, Nk], [0, 32]]))
        # b4y = B4(Yu) [oc,r2,c,r], B[r2,r]=Yu[r,r2] (transposed)
        for (qoff, soff, sgn) in ((0, 0, 1.0), (4, 16, -1.0),
                                  (32, 16, 1.0), (36, 0, 1.0)):
            if sgn > 0:
                v.tensor_copy(av(b4y, qoff, [[64, Nk], [8, 4], [1, 4]]),
                              av(Yu32, soff, [[32, Nk], [1, 4], [4, 4]]))
            else:
                v.tensor_scalar_mul(av(b4y, qoff, [[64, Nk], [8, 4], [1, 4]]),
                                    av(Yu32, soff, [[32, Nk], [1, 4], [4, 4]]), -1.0)

        if dbg is not None:
            nc.sync.dma_start(out=dbg["Y32"][hs:hs + P], in_=Y32)

        # ---------- pass 3: V, C, D, E, W, mgf, eg per k-chunk ----------
        for ch in range(NCH):
            k0 = ch * KC

            # V = T' @ Yu: A=a3ve [g,c,r], B4=b4y; d1=g,d2=r2,j=r
            for kk in range(KC):
                cmat(a3ve, (k0 + kk) * 128, 8, 4, b4y, (k0 + kk) * 64, 32, 8, 4,
                     Ng, Nrf, Nrf, p_off=kk * 1024)
            ktv, kov = fold_reduce(Pt, 0, KC * 2 * Ng * Nrf, 8)
            kt, ko = ktv, kov
            # K [kk,outc,g,r2] strides 128,64,4,1 -> b4cw [oc,r2,c,g'](V)
            v.tensor_copy(av(b4cw, 0, [[256, KC], [32, 4], [1, 16]]),
                          av(kt, ko, [[128, KC], [1, 4], [4, 16]]))
            v.tensor_scalar_mul(av(b4cw, 16, [[256, KC], [32, 4], [1, 16]]),
                                av(kt, ko + 64, [[128, KC], [1, 4], [4, 16]]), -1.0)
            v.tensor_copy(av(b4cw, 128, [[256, KC], [32, 4], [1, 16]]),
                          av(kt, ko + 64, [[128, KC], [1, 4], [4, 16]]))
            v.tensor_copy(av(b4cw, 144, [[256, KC], [32, 4], [1, 16]]),
                          av(kt, ko, [[128, KC], [1, 4], [4, 16]]))

            # W = Lam @ V (Gauss): B3(V) slots (Vre,Vim-Vre,Vre+Vim) [s,r2,g']
            v.tensor_copy(av(b3w, 0, [[192, KC], [16, 4], [1, 16]]),
                          av(ktv, kov, [[128, KC], [1, 4], [4, 16]]))
            v.tensor_sub(av(b3w, 64, [[192, KC], [16, 4], [1, 16]]),
                         av(ktv, kov + 64, [[128, KC], [1, 4], [4, 16]]),
                         av(ktv, kov, [[128, KC], [1, 4], [4, 16]]))
            v.tensor_add(av(b3w, 128, [[192, KC], [16, 4], [1, 16]]),
                         av(ktv, kov, [[128, KC], [1, 4], [4, 16]]),
                         av(ktv, kov + 64, [[128, KC], [1, 4], [4, 16]]))

            if dbg is not None:
                nc.sync.dma_start(out=dbg["vw"][hs:hs + P, k0 * 256:(k0 + KC) * 256],
                                  in_=b4cw)

            # C = T'^H @ V: A=a3c [r1,c,g'], B4=b4cw; d1=r1,d2=r2,j=g'
            for kk in range(KC):
                cmat(a3c, (k0 + kk) * 128, 32, 16, b4cw, kk * 256, 128, 32, 16,
                     Nrf, Nrf, Ng, p_off=kk * 1024)
            kt, ko = fold_reduce(Pt, 0, KC * 2 * Nrf * Nrf, 32)
            v.tensor_copy(av(C32, 0, [[32, KC], [16, 2], [1, 16]]),
                          av(kt, ko, [[32, KC], [16, 2], [1, 16]]))

            # D = u * Y @ C (f32). product dims per k: [r1, r2, m]
            for comp in (0, 1):
                for kk in range(KC):
                    yv = lambda c: av(Y32, (k0 + kk) * 32 + c * 16,
                                      [[4, 4], [0, 4], [1, 4]])
                    cv = lambda c: av(C32, kk * 32 + c * 16,
                                      [[0, 4], [1, 4], [4, 4]])
                    t0 = av(sc0, kk * 64, [[16, 4], [4, 4], [1, 4]])
                    t1 = av(sc0, KC * 64 + kk * 64, [[16, 4], [4, 4], [1, 4]])
                    v.tensor_mul(t0, yv(0), cv(comp))
                    v.tensor_mul(t1, yv(1), cv(1 - comp))
                t0f = av(sc0, 0, [[1, KC * 64]])
                t1f = av(sc0, KC * 64, [[1, KC * 64]])
                if comp == 0:
                    v.tensor_sub(t0f, t0f, t1f)
                else:
                    v.tensor_add(t0f, t0f, t1f)
                v.tensor_reduce(av(D32, comp * 16, [[32, KC], [1, 16], [0, 1]]),
                                av(sc0, 0, [[64, KC], [4, 16], [1, 4]]),
                                axis=AX, op=ADD)
            v.tensor_mul(av(D32, 0, [[32, KC], [1, 32]]),
                         av(D32, 0, [[32, KC], [1, 32]]),
                         av(u_t, k0, [[1, KC], [0, 32]]))
            # b4e = B4(D) [oc,r2,c,r], B[r2,r]=D[r,r2]
            for (qoff, soff, sgn) in ((0, 0, 1.0), (4, 16, -1.0),
                                      (32, 16, 1.0), (36, 0, 1.0)):
                if sgn > 0:
                    v.tensor_copy(av(b4e, qoff, [[64, KC], [8, 4], [1, 4]]),
                                  av(D32, soff, [[32, KC], [1, 4], [4, 4]]))
                else:
                    v.tensor_scalar_mul(av(b4e, qoff, [[64, KC], [8, 4], [1, 4]]),
                                        av(D32, soff, [[32, KC], [1, 4], [4, 4]]), -1.0)

            # E = T' @ D: A=a3ve, B4=b4e; d1=g,d2=r2,j=r
            for kk in range(KC):
                cmat(a3ve, (k0 + kk) * 128, 8, 4, b4e, kk * 64, 32, 8, 4,
                     Ng, Nrf, Nrf, p_off=kk * 1024)
            kt, ko = fold_reduce(Pt, 0, KC * 2 * Ng * Nrf, 8)
            v.tensor_copy(ke[:, 0:KC * 128], av(kt, ko, [[1, KC * 128]]))

            # (B3(V) for W built right after V fold, above)
            for kk in range(KC):
                cmat3(lam_t, (k0 + kk) * WLAM, 48, 16, b3w, kk * 192, 64, 16,
                      Ng, Nrf, Ng, p_off=kk * 3072)
            kt, ko = fold_reduce(Pt, 0, KC * 3 * Ng * Nrf, 16)
            # mgf = W - E into mgf_t [g,cc,r,oc]: Wre=K1-K3, Wim=K1+K2
            # (0,0)=(1,1)=mre=Wre-KE0; (0,1)=mim=Wim-KE1; (1,0)=-mim
            for kk in range(KC):
                moff = (k0 + kk) * 256
                kbw = ko + kk * 192
                kbe = kk * 128
                d9 = av(mgf_t, moff, [[9, 2], [16, 16], [2, 4]])
                v.tensor_sub(d9, av(kt, kbw, [[0, 2], [4, 16], [1, 4]]),
                             av(kt, kbw + 128, [[0, 2], [4, 16], [1, 4]]))
                v.tensor_sub(d9, d9, av(ke, kbe, [[0, 2], [4, 16], [1, 4]]))
                d1v = av(mgf_t, moff + 1, [[16, 16], [2, 4]])
                v.tensor_add(d1v, av(kt, kbw, [[4, 16], [1, 4]]),
                             av(kt, kbw + 64, [[4, 16], [1, 4]]))
                v.tensor_sub(d1v, d1v, av(ke, kbe + 64, [[4, 16], [1, 4]]))
                d8 = av(mgf_t, moff + 8, [[16, 16], [2, 4]])
                v.tensor_sub(d8, av(ke, kbe + 64, [[4, 16], [1, 4]]),
                             av(kt, kbw, [[4, 16], [1, 4]]))
                v.tensor_sub(d8, d8, av(kt, kbw + 64, [[4, 16], [1, 4]]))

        # ---------- PE: eg = sum_k Gh @ mgf ----------
        # mgf_t -> mstg (k-major), then gather stationary sta:
        # rows 32*kq+2g+cc (4 k per matmul), cols (sample*4+kgrp)*8+roc
        nc.sync.dma_start(out=mstg[h].rearrange("k p w -> p k w"),
                          in_=mgf_t.rearrange("p (k w) -> p k w", k=Nk))
        sta = blk.tile([P, 4096], F16, name="sta")
        m0 = mstg[h, 0]
        for kq in range(4):
            for kgrp in range(4):
                nc.sync.dma_start(
                    out=bass.AP(tensor=sta.tensor,
                                offset=sta.offset + 32 * kq * 4096 + kgrp * 8,
                                ap=[[4096, 32], [32, P], [1, 8]]),
                    in_=bass.AP(tensor=m0.tensor,
                                offset=m0.offset + (kgrp * 4 + kq) * P * 256,
                                ap=[[8, 32], [256, P], [1, 8]]))
        # moving: separate tile so next block's GF can overlap this tail
        g3mov = blk.tile([P, 64 * Nk * Ng], F16, name="g3mov")
        nc.sync.dma_start(out=g3mov, in_=g3[h])
        egsb = blk.tile([P, 1024], F32, name="egsb")
        for sgrp in range(32):
            pt = psum.tile([P, 32], F32, name="peg")
            for sq in range(4):
                s = sgrp * 4 + sq
                for kgrp in range(4):
                    nc.tensor.matmul(
                        out=pt[32 * sq:32 * sq + 8, :],
                        lhsT=av(sta, (s * 4 + kgrp) * 8, [[1, 8]]),
                        rhs=av(g3mov, (s * 4 + kgrp) * 32, [[1, 32]]),
                        start=(kgrp == 0), stop=(kgrp == 3),
                        tile_position=(0, 32 * sq))
            nc.scalar.activation(egsb[:, sgrp * 32:(sgrp + 1) * 32], pt, AF.Copy)
        nc.sync.dma_start(out=estg[h], in_=egsb)
        # gather eg to sample-major egT [p, (c,n,r)]: per (oc, s4)
        e0 = estg[h]
        for oc in (0, 1):
            for s4 in range(4):
                nc.sync.dma_start(
                    out=bass.AP(tensor=egT.tensor,
                                offset=egT.offset + s4 * 256 + oc * 128,
                                ap=[[1024, 32], [4, 32], [1, 4]]),
                    in_=bass.AP(tensor=e0.tensor,
                                offset=e0.offset + (32 * s4 + oc) * 1024,
                                ap=[[32, 32], [1, 32], [2048, 4]]))

        # ---------- epilogue ----------
        if dbg is not None:
            nc.sync.dma_start(out=dbg["egT"][hs:hs + P], in_=egT)
        egr = egT[:, 0:128]
        egi = egT[:, 128:256]
        v.tensor_scalar_mul(egr, egr, -1.0 / Nk)
        v.tensor_scalar_mul(egi, egi, -1.0 / Nk)
        v.tensor_mul(egr, egr, pm_t)
        v.tensor_mul(egi, egi, pm_t)

        # step MLP (fixed leaky-relu)
        iq = blk.tile([P, 64], F32, name="iq")
        v.tensor_reduce(av(iq, 0, [[1, 32], [0, 1]]),
                        av(f32_t, 0, [[4, 32], [1, 4]]), axis=AX, op=ADD)
        v.tensor_reduce(av(iq, 32, [[1, 32], [0, 1]]),
                        av(f32_t, 128, [[4, 32], [1, 4]]), axis=AX, op=ADD)
        v.tensor_mul(iq, iq, bnsc)
        v.tensor_add(iq, iq, bnsh)
        v.tensor_mul(iq, iq, dw_t)
        z = blk.tile([P, 1], F32, name="z")
        v.tensor_reduce(z, iq.unsqueeze(1), axis=AX, op=ADD)
        v.tensor_add(z, z, db_t)
        smax = blk.tile([P, 1], F32, name="smax")
        step = blk.tile([P, 1], F32, name="step")
        v.tensor_scalar_max(smax, z, 0.0)
        v.tensor_scalar_min(step, z, 0.0)
        v.scalar_tensor_tensor(out=step, in0=step, scalar=ALPHA, in1=smax,
                               op0=MULT, op1=ADD)

        # proj = Re(eg * conj(Frf)); rg = eg - proj*Frf
        proj = blk.tile([P, 128], F32, name="proj")
        t0 = sc1[:, 0:128]
        t1 = sc2[:, 0:128]
        v.tensor_mul(proj, egr, fre)
        v.tensor_mul(t0, egi, fim)
        v.tensor_add(proj, proj, t0)
        rgr = blk.tile([P, 128], F32, name="rgr")
        rgi = blk.tile([P, 128], F32, name="rgi")
        v.tensor_mul(t0, proj, fre)
        v.tensor_sub(rgr, egr, t0)
        v.tensor_mul(t0, proj, fim)
        v.tensor_sub(rgi, egi, t0)

        # nrm; sc = -step/nrm
        n2 = blk.tile([P, 1], F32, name="n2")
        v.tensor_mul(t0, rgr, rgr)
        v.tensor_mul(t1, rgi, rgi)
        v.tensor_add(t0, t0, t1)
        v.tensor_reduce(n2, t0.unsqueeze(1), axis=AX, op=ADD)
        nc.scalar.activation(n2, n2, AF.Sqrt, bias=zero1)
        v.reciprocal(n2, n2)
        v.tensor_mul(n2, n2, step)
        v.tensor_scalar_mul(n2, n2, -1.0)
        fnr = blk.tile([P, 128], F32, name="fnr")
        fni = blk.tile([P, 128], F32, name="fni")
        v.scalar_tensor_tensor(out=fnr, in0=rgr, scalar=n2, in1=fre,
                               op0=MULT, op1=ADD)
        v.scalar_tensor_tensor(out=fni, in0=rgi, scalar=n2, in1=fim,
                               op0=MULT, op1=ADD)

        # scale = relu(|fnew|-1)+1 ; out = fnew/scale (interleaved)
        m2 = blk.tile([P, 128], F32, name="m2")
        v.tensor_mul(m2, fnr, fnr)
        v.tensor_mul(t0, fni, fni)
        v.tensor_add(m2, m2, t0)
        nc.scalar.activation(m2, m2, AF.Sqrt, bias=zero1)
        nc.scalar.activation(m2, m2, AF.Relu, bias=neg1)
        v.tensor_scalar_add(m2, m2, 1.0)
        v.reciprocal(m2, m2)
        ob = blk.tile([P, 256], F32, name="ob")
        v.tensor_mul(av(ob, 0, [[2, 128]]), fnr, m2)
        v.tensor_mul(av(ob, 1, [[2, 128]]), fni, m2)
        nc.sync.dma_start(out=out_v[hs:hs + P], in_=ob)


def inv4_batched(v, scr, S32, Y32):
    """Y32 = inv(S32) for Nk batched 4x4 complex mats, layout [k,c,i,j] f32.
    2x2 block Schur: S=[[A,B],[C,D]] -> iA, E=D-C iA B, iE, assemble."""
    KS = 32  # per-k stride

    # mini complex 2x2 tiles: layout [k, c(4), i(2), j(1)] width Nk*8
    mt = {nm: scr.tile([P, Nk * 8], F32, name="m_" + nm)
          for nm in ("iA", "iE", "P2", "Q2", "T2", "E2")}
    sc = scr.tile([P, Nk * 16], F32, name="m_sc")

    def sl(i):  # scratch slot [p, Nk]
        return av(sc, i * Nk, [[1, Nk]])

    def ent(t, base, c, i, j, mini=False):
        if mini:
            return av(t, c * 4 + i * 2 + j, [[8, Nk]])
        return av(t, base + c * 16 + i * 4 + j, [[KS, Nk]])

    def blkv(t, base, c, mini=False):
        if mini:
            return av(t, c * 4, [[8, Nk], [2, 2], [1, 2]])
        return av(t, base + c * 16, [[KS, Nk], [4, 2], [1, 2]])

    def c2inv(dst, src, sbase, smini):
        """dst (mini) = inv of 2x2 complex block of src at sbase."""
        e = lambda c, i, j: ent(src, sbase, c, i, j, smini)
        dre, dim, q0, q1 = sl(0), sl(1), sl(2), sl(3)
        # det = s00*s11 - s01*s10
        v.tensor_mul(dre, e(0, 0, 0), e(0, 1, 1))
        v.tensor_mul(q0, e(1, 0, 0), e(1, 1, 1))
        v.tensor_sub(dre, dre, q0)
        v.tensor_mul(q0, e(0, 0, 1), e(0, 1, 0))
        v.tensor_sub(dre, dre, q0)
        v.tensor_mul(q0, e(1, 0, 1), e(1, 1, 0))
        v.tensor_add(dre, dre, q0)
        v.tensor_mul(dim, e(0, 0, 0), e(1, 1, 1))
        v.tensor_mul(q0, e(1, 0, 0), e(0, 1, 1))
        v.tensor_add(dim, dim, q0)
        v.tensor_mul(q0, e(0, 0, 1), e(1, 1, 0))
        v.tensor_sub(dim, dim, q0)
        v.tensor_mul(q0, e(1, 0, 1), e(0, 1, 0))
        v.tensor_sub(dim, dim, q0)
        # inv det (conj form)
        v.tensor_mul(q0, dre, dre)
        v.tensor_mul(q1, dim, dim)
        v.tensor_add(q0, q0, q1)
        v.reciprocal(q0, q0)
        v.tensor_mul(dre, dre, q0)
        v.tensor_mul(dim, dim, q0)
        v.tensor_scalar_mul(dim, dim, -1.0)
        # adj entries into sc slots 4..11: [a11,-a01,-a10,a00] per comp
        for c in (0, 1):
            base_sl = 4 + c * 4
            v.tensor_copy(sl(base_sl + 0), e(c, 1, 1))
            v.tensor_scalar_mul(sl(base_sl + 1), e(c, 0, 1), -1.0)
            v.tensor_scalar_mul(sl(base_sl + 2), e(c, 1, 0), -1.0)
            v.tensor_copy(sl(base_sl + 3), e(c, 0, 0))
        adjr = av(sc, 4 * Nk, [[1, Nk], [2 * Nk, 2], [Nk, 2]])
        adji = av(sc, 8 * Nk, [[1, Nk], [2 * Nk, 2], [Nk, 2]])
        q2 = av(sc, 12 * Nk, [[1, Nk], [2 * Nk, 2], [Nk, 2]])
        dreb = av(sc, 0, [[1, Nk], [0, 2], [0, 2]])
        dimb = av(sc, Nk, [[1, Nk], [0, 2], [0, 2]])
        dr = blkv(dst, 0, 0, True)
        di = blkv(dst, 0, 1, True)
        # re = adjr*dre - adji*dim ; im = adjr*dim + adji*dre
        v.tensor_mul(q2, adjr, dreb)
        v.tensor_mul(dr, adji, dimb)
        v.tensor_sub(dr, q2, dr)
        v.tensor_mul(q2, adjr, dimb)
        v.tensor_mul(di, adji, dreb)
        v.tensor_add(di, q2, di)

    def c2mul(dst, x, xbase, xmini, y, ybase, ymini,
              sub_from=None, sf_base=0, sf_mini=False, negate=False):
        """dst (mini) = x@y | sub_from - x@y | -(x@y), 2x2 complex."""
        t0f = av(sc, 0, [[1, Nk], [Nk, 8]])
        t1f = av(sc, 8 * Nk, [[1, Nk], [Nk, 8]])

        def bxr(t, base, c, i, mini):
            # row i of x block: [k, j(bcast), m]
            if mini:
                return av(t, c * 4 + i * 2, [[8, Nk], [0, 2], [1, 2]])
            return av(t, base + c * 16 + i * 4, [[KS, Nk], [0, 2], [1, 2]])

        def by(t, base, c, mini):
            # y block as [k, j, m] with entry (m,j)
            if mini:
                return av(t, c * 4, [[8, Nk], [1, 2], [2, 2]])
            return av(t, base + c * 16, [[KS, Nk], [1, 2], [4, 2]])

        for comp in (0, 1):
            for i in (0, 1):
                t0r = av(sc, i * 4 * Nk, [[1, Nk], [2 * Nk, 2], [Nk, 2]])
                t1r = av(sc, (8 + i * 4) * Nk, [[1, Nk], [2 * Nk, 2], [Nk, 2]])
                v.tensor_mul(t0r, bxr(x, xbase, 0, i, xmini), by(y, ybase, comp, ymini))
                v.tensor_mul(t1r, bxr(x, xbase, 1, i, xmini), by(y, ybase, 1 - comp, ymini))
            if comp == 0:
                v.tensor_sub(t0f, t0f, t1f)
            else:
                v.tensor_add(t0f, t0f, t1f)
            d = blkv(dst, 0, comp, True)
            red = av(sc, 8 * Nk, [[1, Nk], [Nk, 4], [0, 1]])
            v.tensor_reduce(red, av(sc, 0, [[1, Nk], [2 * Nk, 4], [Nk, 2]]),
                            axis=AX, op=ADD)
            redv = av(sc, 8 * Nk, [[1, Nk], [2 * Nk, 2], [Nk, 2]])
            if sub_from is not None:
                v.tensor_sub(d, blkv(sub_from, sf_base, comp, sf_mini), redv)
            elif negate:
                v.tensor_scalar_mul(d, redv, -1.0)
            else:
                v.tensor_copy(d, redv)

    iA, iE, P2, Q2, T2, E2 = (mt[n] for n in ("iA", "iE", "P2", "Q2", "T2", "E2"))
    # S blocks: A off 0, B off 2, C off 8, D off 10
    c2inv(iA, S32, 0, False)
    c2mul(P2, iA, 0, True, S32, 2, False)                 # P2 = iA@B
    c2mul(Q2, S32, 8, False, iA, 0, True)                 # Q2 = C@iA
    c2mul(E2, S32, 8, False, P2, 0, True,
          sub_from=S32, sf_base=10, sf_mini=False)        # E2 = D - C@P2
    c2inv(iE, E2, 0, True)
    c2mul(T2, P2, 0, True, iE, 0, True, negate=True)      # T2 = -P2@iE = Y12
    # write Y12 -> Y32[0:2,2:4]
    for c in (0, 1):
        v.tensor_copy(blkv(Y32, 2, c), blkv(T2, 0, c, True))
    c2mul(E2, iE, 0, True, Q2, 0, True, negate=True)      # E2 = -iE@Q2 = Y21
    for c in (0, 1):
        v.tensor_copy(blkv(Y32, 8, c), blkv(E2, 0, c, True))
    # Y11 = iA - Y12@Q2  (T2 holds Y12; write to E2 to avoid dst/src alias)
    c2mul(E2, T2, 0, True, Q2, 0, True,
          sub_from=iA, sf_base=0, sf_mini=True)
    for c in (0, 1):
        v.tensor_copy(blkv(Y32, 0, c), blkv(E2, 0, c, True))
    for c in (0, 1):
        v.tensor_copy(blkv(Y32, 10, c), blkv(iE, 0, c, True))


# ================= host side =================

_NC_CACHE = {}


def _prep(inputs):
    f16, f32 = np.float16, np.float32
    Gr = np.asarray(inputs["G_re"], f32)
    Gi = np.asarray(inputs["G_im"], f32)
    Lr = np.asarray(inputs["Lam_re"], f32)
    Li = np.asarray(inputs["Lam_im"], f32)
    fr = np.asarray(inputs["Frf_re"], f32)
    fi = np.asarray(inputs["Frf_im"], f32)


    lamx = np.empty((Nk, B, Ng, 3, Ng), f16)    # Lam 3-slot: [g,s,g']
    lamx[..., 0, :] = Lr + Li
    lamx[..., 1, :] = Lr
    lamx[..., 2, :] = Li
    frT = fr.transpose(0, 2, 1)                  # [B,r,n]
    fiT = fi.transpose(0, 2, 1)
    fb4 = np.empty((B, 2, Nrf, 2, Nt), f16)     # B4(Frf): [oc,r,c,n]
    fb4[:, 0, :, 0, :] = frT
    fb4[:, 0, :, 1, :] = -fiT
    fb4[:, 1, :, 0, :] = fiT
    fb4[:, 1, :, 1, :] = frT
    f32x = np.empty((B, 2, Nt, Nrf), f32)
    f32x[:, 0] = fr
    f32x[:, 1] = fi
    u32 = np.ascontiguousarray(
        (1.0 / np.asarray(inputs["Beta_re"], f32).reshape(Nk, B).T))
    aux = np.concatenate([
        np.asarray(inputs["bn_gamma"], f32).ravel(),
        np.asarray(inputs["bn_beta"], f32).ravel(),
        np.asarray(inputs["bn_mean"], f32).ravel(),
        np.asarray(inputs["bn_var"], f32).ravel(),
        np.asarray(inputs["dense_w"], f32).ravel(),
        np.asarray(inputs["dense_b"], f32).ravel(),
        np.asarray(inputs["P_mask"], f32).ravel()])

    in_maps = []
    for c in range(NCORES):
        s = slice(c * BL, (c + 1) * BL)
        # gpe: moving G for PE-GF: [h, row=64s+32cc+n, pair*256 + k*16+g]
        Xc = np.stack([Gr[:, s], Gi[:, s]])          # [cc,k,BL,n,g]
        Xc = Xc.reshape(2, Nk, H, 64, 2, Nt, Ng)     # [cc,k,h,q,ss,n,g]
        gpe = np.ascontiguousarray(
            Xc.transpose(2, 4, 0, 5, 3, 1, 6).reshape(H, P, 64 * Nk * Ng)
        ).astype(f16)
        # fpe: block-diag stationary: rows as gpe, cols pair*16 + 8s+2r+oc
        # oc=0 (re): (cc0: fr, cc1: fi) ; oc=1 (-im, conj): (cc0: -fi, cc1: fr)
        frc = fr[s].reshape(H, 64, 2, Nt, Nrf)       # [h,q,ss,n,r]
        fic = fi[s].reshape(H, 64, 2, Nt, Nrf)
        fpe_a = np.zeros((H, 2, 2, Nt, 64, 16), np.float32)  # [h,ss,cc,n,q,col]
        for ss in (0, 1):
            fq = frc[:, :, ss].transpose(0, 2, 1, 3)  # [h,n,q,r]
            gq = fic[:, :, ss].transpose(0, 2, 1, 3)
            cb = np.empty((2, H, Nt, 64, Nrf, 2), np.float32)  # [cc,h,n,q,r,oc]
            cb[0, ..., 0] = fq
            cb[0, ..., 1] = -gq
            cb[1, ..., 0] = gq
            cb[1, ..., 1] = fq
            for cc in (0, 1):
                fpe_a[:, ss, cc, :, :, 8 * ss:8 * ss + 8] = \
                    cb[cc].reshape(H, Nt, 64, 8)
        fpe = np.ascontiguousarray(fpe_a.reshape(H, P, 64 * 16)).astype(f16)
        # g3: eg moving operand: rows 32*kq+2g+cc, cols (sample*4+kgrp)*32+n
        Yc = np.stack([Gr[:, s], Gi[:, s]])          # [cc,k,BL,n,g]
        Yc = Yc.reshape(2, 4, 4, H, P, Nt, Ng)       # [cc,kgrp,kq,h,smp,n,g]
        g3 = np.ascontiguousarray(
            Yc.transpose(3, 2, 6, 0, 4, 1, 5).reshape(H, P, 32 * Nk * Nt)
        ).astype(f16)
        in_maps.append({
            "gpe": gpe,
            "fpe": fpe,
            "g3": g3,
            "lam": np.ascontiguousarray(lamx[:, s]).reshape(Nk, BL, WLAM),
            "fb4": np.ascontiguousarray(fb4[s]).reshape(BL, 512),
            "f32t": np.ascontiguousarray(f32x[s]).reshape(BL, 256),
            "u32": np.ascontiguousarray(u32[s]),
            "aux": aux,
        })
    return in_maps


def kernel(**inputs):
    in_maps = _prep(inputs)
    if "nc" not in _NC_CACHE:
        _NC_CACHE["nc"] = build_nc()
    nc = _NC_CACHE["nc"]
    from concourse.bass_utils import run_bass_kernel_spmd
    res = run_bass_kernel_spmd(nc, in_maps, core_ids=list(range(NCORES)))
    outs = [res.results[c]["out"] for c in range(NCORES)]
    full = np.concatenate(outs, axis=0).reshape(B, Nt, Nrf, 2)
    return np.ascontiguousarray(full).view(np.complex64).reshape(B, Nt, Nrf)


def kernel_profiled(**inputs):
    """Returns modeled HW exec time (ns) from the cost-model timeline sim."""
    if "nc" not in _NC_CACHE:
        _NC_CACHE["nc"] = build_nc()
    from concourse.timeline_sim import TimelineSim
    ts = TimelineSim(_NC_CACHE["nc"], no_exec=True)
    return int(ts.simulate())
